# revision 1
# baseline (speedup 1.0000x reference)
"""DYSPN attention-conv kernel v2 for Trainium2 (8 NeuronCores, batch-parallel).

Math (unfold/fold pair collapses algebraically; see derivation):
  per image, tap k=(i,j) != center, ring r = INDEX[i,j], dy = 3-i, dx = 3-j:
    z_k[y,x] = att_r[y,x] * aff_k[y,x]
    U[y,x]   = sum_k z_k[y,x]
    D[y,x]   = sum_k (|z_k| - z_k)[y,x] = sum_k 2*relu(-z_k)   (att >= 0)
    T[y,x]   = sum_k z_k[y+dy, x+dx]  (zero outside image)
  out = ((T + att3)*cs + D*co) / (U + D + att3 + eps)
  (identical to the reference: A = U + D, denom = A + att3 + eps,
   out = (T+att3)*cs/denom + ((denom - U - att3)/denom)*co, and
   denom - U - att3 = D + eps)

Implementation strategy (per core: 2 images, 4 blocks of 128 rows):
  - Host: cast aff/att/cs to bf16; pack aff as guard-padded planes
    [img, blk, part(row), tap(ring-sorted), 264] so DMA lands compute-ready
    (no transposes, no guard memsets).  HBM traffic halves vs fp32.
  - DVE: z = att*aff (bf16 TT 2x, 3 ring-broadcast instrs/block), pre-fold
    of ring0 halves for the U and D reductions, fp32 epilogue.
  - ACT/POOL/DVE split: d = relu(-z) planes (tensor_scalar(min 0, mult -1) /
    activation(Relu, scale=-1)).
  - PE: U/D/T reductions as bf16 banded-identity matmuls, 2 taps per matmul
    into 2-wide PSUM accumulators [128,2,256] (one bank), folded in the
    epilogue.  T row-shifts via band-diagonal offsets, column shifts baked
    into custom moving APs (stride 264 + dx2-dx1); cross-block halo rows via
    off-diagonal band windows.
"""
import sys

sys.path.insert(0, "/opt/trn_rl_repo")

import numpy as np
import ml_dtypes

import concourse.bass as bass  # noqa: F401
import concourse.tile as tile
from concourse import bacc, mybir
from concourse.ap import AP
from concourse.bass_utils import run_bass_kernel_spmd

BF16 = mybir.dt.bfloat16
FP32 = mybir.dt.float32

N_CORES = 8
B_FULL = 16
B_CORE = B_FULL // N_CORES
H = W = 256
K = 7
GW = 4                    # zero guard cols each side (host-packed)
PW = W + 2 * GW           # 264: plane pitch in the z tile
NTAP = 48
BANDW = 390
C0 = 131                  # identity diagonal column offset in band1
ID2 = BANDW               # ident2 (2.0 diagonal) starts at this column
EPS = 1e-6

_INDEX = np.array([0, 0, 0, 0, 0, 0, 0,
                   0, 1, 1, 1, 1, 1, 0,
                   0, 1, 2, 2, 2, 1, 0,
                   0, 1, 2, 3, 2, 1, 0,
                   0, 1, 2, 2, 2, 1, 0,
                   0, 1, 1, 1, 1, 1, 0,
                   0, 0, 0, 0, 0, 0, 0], dtype=np.int64).reshape(K, K)

# ring-major, row-minor, col-minor tap order; t = SBUF/DRAM plane index
TAPORD = [(i, j) for r in (0, 1, 2) for i in range(K) for j in range(K)
          if (i, j) != (3, 3) and _INDEX[i, j] == r]
RING_SEG = [(0, 24, 0), (24, 40, 1), (40, 48, 2)]   # t-ranges per ring
# mult instr ranges (ring0 split at row boundary t=11 for DMA overlap)
MULT_RUNS = [(0, 11, 0), (11, 24, 0), (24, 40, 1), (40, 48, 2)]
DMA_CHUNKS = [(24, 40), (40, 48), (0, 11), (11, 24)]

# T groups: maximal t-contiguous runs sharing row i (same dy)
TGROUPS = []  # (t_lo, t_hi, i)
for t, (i, j) in enumerate(TAPORD):
    if TGROUPS and TGROUPS[-1][2] == i and TGROUPS[-1][1] == t:
        TGROUPS[-1][1] = t + 1
    else:
        TGROUPS.append([t, t + 1, i])
TGROUPS = [tuple(g) for g in TGROUPS]


def dxof(t):
    return 3 - TAPORD[t][1]


def dyof(t):
    return 3 - TAPORD[t][0]


def _chunk_of(t):
    for ci, (lo, hi) in enumerate(DMA_CHUNKS):
        if lo <= t < hi:
            return ci
    raise AssertionError


def band_np() -> np.ndarray:
    b = np.zeros((128, BANDW + 128), dtype=np.float32)
    for p in range(128):
        b[p, p + C0] = 1.0
        b[p, ID2 + p] = 2.0
    return b.astype(ml_dtypes.bfloat16)


def _to_bf16_round(x: np.ndarray) -> np.ndarray:
    """fp32 -> bf16 with round-to-nearest-even, fast numpy path."""
    u = x.view(np.uint32)
    r = ((u >> 16) & 1).astype(np.uint32)
    out = ((u + 0x7FFF + r) >> 16).astype(np.uint16)
    return out.view(ml_dtypes.bfloat16)


def pack_inputs(aff, att, cs, co):
    """Full fp32 inputs -> per-core input maps (host-side layout/cast only)."""
    B = B_FULL
    # affinity: [B,49,H,W] -> guarded bf16 planes [B,2,128,48,264] in TAPORD
    kidx = np.array([i * K + j for (i, j) in TAPORD])
    aff_sel = aff[:, kidx]                             # [B,48,H,W]
    aff_bf = _to_bf16_round(np.ascontiguousarray(aff_sel))
    packed = np.zeros((B, 2, 128, NTAP, PW), dtype=ml_dtypes.bfloat16)
    a = aff_bf.reshape(B, NTAP, 2, 128, W).transpose(0, 2, 3, 1, 4)
    packed[:, :, :, :, GW:GW + W] = a
    att_bf = _to_bf16_round(np.ascontiguousarray(att))  # [B,4,H,W]
    att_p = att_bf.reshape(B, 4, 2, 128, W).transpose(0, 2, 3, 1, 4)
    att_p = np.ascontiguousarray(att_p)                 # [B,2,128,4,W]
    cs32 = np.ascontiguousarray(cs, dtype=np.float32).reshape(B, 2, 128, W)
    co32 = np.ascontiguousarray(co, dtype=np.float32).reshape(B, 2, 128, W)
    band = band_np()

    in_maps = []
    for c in range(N_CORES):
        s = slice(c * B_CORE, (c + 1) * B_CORE)
        in_maps.append({
            "aff": np.ascontiguousarray(packed[s]),
            "att": np.ascontiguousarray(att_p[s]),
            "cs": np.ascontiguousarray(cs32[s]),
            "co": np.ascontiguousarray(co32[s]),
            "band": band,
        })
    return in_maps


def _pair_ap(zt, t1, t2, w1, w2):
    """Custom moving AP over taps {t1,t2} with per-tap column windows."""
    base = zt[:]
    pstride = base.ap[0][0]
    off = base.offset + t1 * PW + w1
    stride = (t2 - t1) * PW + (w2 - w1)
    return AP(base.tensor, off, [[pstride, 128], [stride, 2], [1, W]])


def _single_ap(zt, t, w):
    base = zt[:]
    pstride = base.ap[0][0]
    return AP(base.tensor, base.offset + t * PW + w, [[pstride, 128], [1, W]])


def _build():
    nc = bacc.Bacc("TRN2", target_bir_lowering=False, debug=False,
                   num_devices=N_CORES)
    aff = nc.dram_tensor("aff", [B_CORE, 2, 128, NTAP, PW], BF16,
                         kind="ExternalInput").ap()
    att = nc.dram_tensor("att", [B_CORE, 2, 128, 4, W], BF16,
                         kind="ExternalInput").ap()
    cs = nc.dram_tensor("cs", [B_CORE, 2, 128, W], FP32,
                        kind="ExternalInput").ap()
    co = nc.dram_tensor("co", [B_CORE, 2, 128, W], FP32,
                        kind="ExternalInput").ap()
    band = nc.dram_tensor("band", [128, BANDW + 128], BF16,
                          kind="ExternalInput").ap()
    out = nc.dram_tensor("out", [B_CORE, 1, H, W], FP32,
                         kind="ExternalOutput").ap()

    with tile.TileContext(nc) as tc:
        with tc.tile_pool(name="const", bufs=1) as cpool, \
             tc.tile_pool(name="inp", bufs=4) as ipool, \
             tc.tile_pool(name="zp", bufs=4) as zpool, \
             tc.tile_pool(name="dp", bufs=2) as dpool, \
             tc.tile_pool(name="fp", bufs=2) as fpool, \
             tc.tile_pool(name="ep", bufs=2) as epool, \
             tc.tile_pool(name="ps", bufs=2, space="PSUM") as pspool:

            bandt = cpool.tile([128, BANDW + 128], BF16)
            nc.scalar.dma_start(out=bandt[:], in_=band[:, :])
            ident = bandt[:, C0:C0 + 128]
            ident2 = bandt[:, ID2:ID2 + 128]

            for img in range(B_CORE):
                # ---- input DMAs ----
                zts, attfs, csts, cots = [], [], [], []
                for b in range(2):
                    attf = ipool.tile([128, 4, W], BF16, tag="attf")
                    nc.scalar.dma_start(out=attf[:], in_=att[img, b])
                    cst = ipool.tile([128, W], FP32, tag="cst")
                    nc.scalar.dma_start(out=cst[:], in_=cs[img, b])
                    cot = ipool.tile([128, W], FP32, tag="cot")
                    nc.scalar.dma_start(out=cot[:], in_=co[img, b])
                    attfs.append(attf)
                    csts.append(cst)
                    cots.append(cot)
                    zt = zpool.tile([128, NTAP, PW], BF16, tag="zt")
                    zts.append(zt)
                for ci, (lo, hi) in enumerate(DMA_CHUNKS):
                    for b in range(2):
                        nc.sync.dma_start(out=zts[b][:, lo:hi, :],
                                          in_=aff[img, b, :, lo:hi, :])

                # ---- PSUM accumulators (one bank each) ----
                psU = [pspool.tile([128, 2, W], FP32, tag="U", name=f"psU{_b}")
                       for _b in range(2)]
                psD = [pspool.tile([128, 2, W], FP32, tag="D", name=f"psD{_b}")
                       for _b in range(2)]
                psT = [pspool.tile([128, 2, W], FP32, tag="T", name=f"psT{_b}")
                       for _b in range(2)]
                started = set()

                def mm(acc, b, stop=False, **kw):
                    key = (acc, b)
                    nc.tensor.matmul(start=(key not in started), stop=stop,
                                     **kw)
                    started.add(key)

                dts = [dpool.tile([128, NTAP, W], BF16, tag="dt",
                                   name=f"dt{_b}") for _b in range(2)]
                # ring0 pre-folds for U (z) and D (d): 24 -> 12 planes
                zus = [fpool.tile([128, 12, W], BF16, tag="zu",
                                   name=f"zu{_b}") for _b in range(2)]
                dus = [fpool.tile([128, 12, W], BF16, tag="du",
                                   name=f"du{_b}") for _b in range(2)]

                # ---- per-chunk compute ----
                for ci, (lo, hi) in enumerate(DMA_CHUNKS):
                    ring = 0 if hi <= 24 else (1 if hi <= 40 else 2)
                    for b in range(2):
                        zt = zts[b]
                        zwin = zt[:, lo:hi, GW:GW + W]
                        # z = att_r * aff  (DVE, in place, bf16 2x)
                        nc.vector.tensor_tensor(
                            out=zwin, in0=zwin,
                            in1=attfs[b][:, ring:ring + 1, :].broadcast_to(
                                [128, hi - lo, W]),
                            op=mybir.AluOpType.mult)
                        # d = relu(-z): split DVE / POOL / ACT
                        dsl = dts[b][:, lo:hi, :]
                        nc.scalar.activation(
                            dsl, zwin, mybir.ActivationFunctionType.Relu,
                            scale=-1.0)

                    for b in range(2):
                        zt = zts[b]
                        if ci == 3:
                            # ring0 folds (need chunks 2+3): U and D inputs
                            nc.vector.tensor_tensor(
                                out=zus[b][:], in0=zt[:, 0:12, GW:GW + W],
                                in1=zt[:, 12:24, GW:GW + W],
                                op=mybir.AluOpType.add)
                            nc.vector.tensor_tensor(
                                out=dus[b][:], in0=dts[b][:, 0:12, :],
                                in1=dts[b][:, 12:24, :],
                                op=mybir.AluOpType.add)
                            for h in range(6):
                                mm("U", b, out=psU[b][:], lhsT=ident,
                                   rhs=zus[b][:, 2 * h:2 * h + 2, :])
                                mm("D", b, stop=(h == 5),
                                   out=psD[b][:], lhsT=ident2,
                                   rhs=dus[b][:, 2 * h:2 * h + 2, :])
                        elif ci <= 1:
                            for t in range(lo, hi, 2):
                                mm("U", b, out=psU[b][:], lhsT=ident,
                                   rhs=zt[:, t:t + 2, GW:GW + W])
                                mm("D", b, out=psD[b][:], lhsT=ident2,
                                   rhs=dts[b][:, t:t + 2, :])
                        # T matmuls for groups inside this chunk
                        for (g_lo, g_hi, i) in TGROUPS:
                            if g_lo < lo or g_lo >= hi:
                                continue
                            dy = 3 - i
                            lw = bandt[:, C0 + dy:C0 + dy + 128]
                            t = g_lo
                            while t + 1 < g_hi:
                                mm("T", b, out=psT[b][:], lhsT=lw,
                                   rhs=_pair_ap(zt, t, t + 1,
                                                GW + dxof(t), GW + dxof(t + 1)))
                                t += 2
                            if t < g_hi:
                                mm("T", b, out=psT[b][:, 0, :], lhsT=lw,
                                   rhs=_single_ap(zt, t, GW + dxof(t)))
                            # halo: b0 rows need dy>0 from b1; b1 need dy<0;
                            # emitted at b==1 so both blocks' z is ready
                            if b == 1 and dy != 0:
                                if dy > 0:
                                    hw = bandt[:, 3 + dy:3 + dy + 128]
                                    dst, other = 0, zts[1]
                                else:
                                    hw = bandt[:, 259 + dy:259 + dy + 128]
                                    dst, other = 1, zts[0]
                                t = g_lo
                                while t + 1 < g_hi:
                                    mm("T", dst, out=psT[dst][:], lhsT=hw,
                                       rhs=_pair_ap(other, t, t + 1,
                                                    GW + dxof(t),
                                                    GW + dxof(t + 1)))
                                    t += 2
                                if t < g_hi:
                                    mm("T", dst, out=psT[dst][:, 0, :],
                                       lhsT=hw,
                                       rhs=_single_ap(other, t, GW + dxof(t)))

                # closers: psU += att3, psT += att3 (stop their groups)
                for b in range(2):
                    mm("U", b, stop=True, out=psU[b][:, 0, :], lhsT=ident,
                       rhs=attfs[b][:, 3, :])
                    mm("T", b, stop=True, out=psT[b][:, 0, :], lhsT=ident,
                       rhs=attfs[b][:, 3, :])

                # ---- epilogue ----
                for b in range(2):
                    # DVE can read only one PSUM operand per op: stage the
                    # second accumulator halves through SBUF on ACT
                    u1 = epool.tile([128, W], FP32, tag="u1")
                    nc.scalar.copy(u1[:], psU[b][:, 1, :])
                    d1 = epool.tile([128, W], FP32, tag="d1")
                    nc.scalar.copy(d1[:], psD[b][:, 1, :])
                    t1 = epool.tile([128, W], FP32, tag="t1")
                    nc.scalar.copy(t1[:], psT[b][:, 1, :])
                    # in-place accumulations to keep the tile count low
                    nc.vector.tensor_tensor(out=u1[:], in0=psU[b][:, 0, :],
                                            in1=u1[:],
                                            op=mybir.AluOpType.add)
                    nc.vector.tensor_tensor(out=d1[:], in0=psD[b][:, 0, :],
                                            in1=d1[:],
                                            op=mybir.AluOpType.add)
                    e = epool.tile([128, W], FP32, tag="e")
                    nc.vector.scalar_tensor_tensor(
                        out=e[:], in0=u1[:], scalar=EPS, in1=d1[:],
                        op0=mybir.AluOpType.add, op1=mybir.AluOpType.add)
                    rcp = epool.tile([128, W], FP32, tag="rcp")
                    nc.vector.reciprocal_approx_fast(out=rcp[:], in_=e[:])
                    nc.vector.tensor_tensor(out=d1[:], in0=d1[:],
                                            in1=cots[b][:],
                                            op=mybir.AluOpType.mult)
                    nc.vector.tensor_tensor(out=t1[:], in0=psT[b][:, 0, :],
                                            in1=t1[:],
                                            op=mybir.AluOpType.add)
                    nc.vector.tensor_tensor(out=t1[:], in0=t1[:],
                                            in1=csts[b][:],
                                            op=mybir.AluOpType.mult)
                    nc.vector.tensor_tensor(out=t1[:], in0=t1[:],
                                            in1=d1[:],
                                            op=mybir.AluOpType.add)
                    outt = epool.tile([128, W], FP32, tag="outt")
                    nc.vector.tensor_tensor(out=outt[:], in0=t1[:],
                                            in1=rcp[:],
                                            op=mybir.AluOpType.mult)
                    nc.sync.dma_start(
                        out=out[img, 0, b * 128:b * 128 + 128, :],
                        in_=outt[:])

    nc.compile()
    return nc


_NC_CACHE = None


def _get_nc():
    global _NC_CACHE
    if _NC_CACHE is None:
        _NC_CACHE = _build()
    return _NC_CACHE


def run(inputs: dict, trace: bool = False):
    aff = np.ascontiguousarray(np.asarray(inputs["affinity"], dtype=np.float32))
    att = np.ascontiguousarray(np.asarray(inputs["attention"], dtype=np.float32))
    cs = np.ascontiguousarray(
        np.asarray(inputs["current_segmentation"], dtype=np.float32))
    co = np.ascontiguousarray(
        np.asarray(inputs["coarse_segmentation"], dtype=np.float32))
    in_maps = pack_inputs(aff, att, cs, co)

    nc = _get_nc()
    last_err = None
    for attempt in range(3):
        try:
            res = run_bass_kernel_spmd(nc, in_maps, list(range(N_CORES)),
                                       trace=trace)
            break
        except Exception as e:
            last_err = e
            import time
            time.sleep(10)
    else:
        raise last_err
    full = np.concatenate([res.results[c]["out"] for c in range(N_CORES)],
                          axis=0)
    return full, res


def kernel(**inputs) -> np.ndarray:
    out, _ = run(inputs, trace=False)
    return out



# revision 2
# speedup vs baseline: 1.0240x; 1.0240x over previous
"""DYSPN attention-conv kernel v3 for Trainium2 (8 NeuronCores, batch-parallel).

Same math as v2, restructured to cut PE work ~40% and rebalance engines:
  per image, tap k=(i,j) != center, ring r = INDEX[i,j], dy = 3-i, dx = 3-j:
    z_k[y,x] = att_r[y,x] * aff_k[y,x]
    U[y,x]   = sum_k z_k[y,x]
    D[y,x]   = sum_k 2*relu(-z_k)[y,x]
    T[y,x]   = sum_k z_k[y+dy, x+dx]  (zero outside image)
  out = ((T + att3)*cs + D*co) / (U + D + att3 + eps)

v3 layout/strategy:
  - z tile holds BOTH 128-row blocks [128, 2, 48, 264]; U/D matmuls pair the
    two blocks in the rhs free dim (512-wide) so psU/psD/psT are one PSUM
    bank each ([128, 2(block), 256]).
  - T: per-row column-shifted sums R_i built on DVE (bf16 2x) via a 3-level
    butterfly (13 instrs/block, affine APs), then only 7 main + 6 halo
    row-shift matmuls per image on PE (vs 90 tap matmuls in v2).
  - D: relu(-z) on ACT into chunked dt tiles, reduced with ident2 matmuls.
  - cs/co shipped as bf16 (halves their HBM traffic).
"""
import sys

sys.path.insert(0, "/opt/trn_rl_repo")

import numpy as np
import ml_dtypes

import concourse.bass as bass  # noqa: F401
import concourse.tile as tile
from concourse import bacc, mybir
from concourse.ap import AP
from concourse.bass_utils import run_bass_kernel_spmd

BF16 = mybir.dt.bfloat16
FP32 = mybir.dt.float32

N_CORES = 8
B_FULL = 16
B_CORE = B_FULL // N_CORES
H = W = 256
K = 7
GW = 4                    # zero guard cols each side (host-packed)
PW = W + 2 * GW           # 264: plane pitch in the z tile
NTAP = 48
BANDW = 390
C0 = 131                  # identity diagonal column offset in band1
ID2 = BANDW               # ident2 (2.0 diagonal) starts at this column
EPS = 1e-6

_INDEX = np.array([0, 0, 0, 0, 0, 0, 0,
                   0, 1, 1, 1, 1, 1, 0,
                   0, 1, 2, 2, 2, 1, 0,
                   0, 1, 2, 3, 2, 1, 0,
                   0, 1, 2, 2, 2, 1, 0,
                   0, 1, 1, 1, 1, 1, 0,
                   0, 0, 0, 0, 0, 0, 0], dtype=np.int64).reshape(K, K)

# ring-major, row-minor, col-minor tap order (same as v2 packing)
TAPORD = [(i, j) for r in (0, 1, 2) for i in range(K) for j in range(K)
          if (i, j) != (3, 3) and _INDEX[i, j] == r]

# z-mult chunks (ring-aligned) and relu/dt chunks (uniform 12)
ZCHUNKS = [(0, 12, (0,)), (12, 24, (0,)), (24, 40, (1,)), (40, 48, (2,))]
RING_OF = {0: (0, 24), 1: (24, 40), 2: (40, 48)}
DCHUNKS = [(0, 12), (12, 24), (24, 36), (36, 48)]

# T row-slot order produced by the butterfly: rows [2,3,4,1,5,0,6]
SLOT_ROWS = [2, 3, 4, 1, 5, 0, 6]
SLOT_DY = [3 - i for i in SLOT_ROWS]   # [1, 0, -1, 2, -2, 3, -3]


def dxof(t):
    return 3 - TAPORD[t][1]


def band_np() -> np.ndarray:
    b = np.zeros((128, BANDW + 128), dtype=np.float32)
    for p in range(128):
        b[p, p + C0] = 1.0
        b[p, ID2 + p] = 2.0
    return b.astype(ml_dtypes.bfloat16)


def _to_bf16_round(x: np.ndarray) -> np.ndarray:
    """fp32 -> bf16 with round-to-nearest-even, fast numpy path."""
    u = x.view(np.uint32)
    r = ((u >> 16) & 1).astype(np.uint32)
    out = ((u + 0x7FFF + r) >> 16).astype(np.uint16)
    return out.view(ml_dtypes.bfloat16)


def pack_inputs(aff, att, cs, co):
    """Full fp32 inputs -> per-core input maps (host-side layout/cast only)."""
    B = B_FULL
    kidx = np.array([i * K + j for (i, j) in TAPORD])
    aff_sel = aff[:, kidx]                             # [B,48,H,W]
    aff_bf = _to_bf16_round(np.ascontiguousarray(aff_sel))
    packed = np.zeros((B, 2, 128, NTAP, PW), dtype=ml_dtypes.bfloat16)
    a = aff_bf.reshape(B, NTAP, 2, 128, W).transpose(0, 2, 3, 1, 4)
    packed[:, :, :, :, GW:GW + W] = a
    att_bf = _to_bf16_round(np.ascontiguousarray(att))  # [B,4,H,W]
    att_p = att_bf.reshape(B, 4, 2, 128, W).transpose(0, 2, 3, 1, 4)
    att_p = np.ascontiguousarray(att_p)                 # [B,2,128,4,W]
    cs_bf = _to_bf16_round(
        np.ascontiguousarray(cs, dtype=np.float32)).reshape(B, 2, 128, W)
    co_bf = _to_bf16_round(
        np.ascontiguousarray(co, dtype=np.float32)).reshape(B, 2, 128, W)
    band = band_np()

    in_maps = []
    for c in range(N_CORES):
        s = slice(c * B_CORE, (c + 1) * B_CORE)
        in_maps.append({
            "aff": np.ascontiguousarray(packed[s]),
            "att": np.ascontiguousarray(att_p[s]),
            "cs": np.ascontiguousarray(cs_bf[s]),
            "co": np.ascontiguousarray(co_bf[s]),
            "band": band,
        })
    return in_maps


def _zap(zt, b, t0, dims):
    """AP over the z tile: start at tap t0 (window GW+dxof(t0)), free dims
    described as (dt, ddx, n) pairs (tap step + window-offset step), with a
    final [1, W] column run."""
    base = zt[:]
    pstride = base.ap[0][0]
    off = base.offset + b * NTAP * PW + t0 * PW + GW + dxof(t0)
    ap = [[pstride, 128]] + [[dt * PW + ddx, n] for (dt, ddx, n) in dims] \
        + [[1, W]]
    return AP(base.tensor, off, ap)


def _pap(tt, b, nplanes, p0, dims):
    """AP over a [128, 2, nplanes, W] staging tile."""
    base = tt[:]
    pstride = base.ap[0][0]
    off = base.offset + b * nplanes * W + p0 * W
    ap = [[pstride, 128]] + [[dp * W, n] for (dp, n) in dims] + [[1, W]]
    return AP(base.tensor, off, ap)


def _build():
    nc = bacc.Bacc("TRN2", target_bir_lowering=False, debug=False,
                   num_devices=N_CORES)
    aff = nc.dram_tensor("aff", [B_CORE, 2, 128, NTAP, PW], BF16,
                         kind="ExternalInput").ap()
    att = nc.dram_tensor("att", [B_CORE, 2, 128, 4, W], BF16,
                         kind="ExternalInput").ap()
    cs = nc.dram_tensor("cs", [B_CORE, 2, 128, W], BF16,
                        kind="ExternalInput").ap()
    co = nc.dram_tensor("co", [B_CORE, 2, 128, W], BF16,
                        kind="ExternalInput").ap()
    band = nc.dram_tensor("band", [128, BANDW + 128], BF16,
                          kind="ExternalInput").ap()
    out = nc.dram_tensor("out", [B_CORE, 1, H, W], FP32,
                         kind="ExternalOutput").ap()

    TT = mybir.AluOpType
    with tile.TileContext(nc) as tc:
        with tc.tile_pool(name="const", bufs=1) as cpool, \
             tc.tile_pool(name="inp", bufs=2) as ipool, \
             tc.tile_pool(name="zp", bufs=2) as zpool, \
             tc.tile_pool(name="dp", bufs=2) as dpool, \
             tc.tile_pool(name="st", bufs=1) as spool, \
             tc.tile_pool(name="rp", bufs=2) as rpool, \
             tc.tile_pool(name="ep", bufs=1) as epool, \
             tc.tile_pool(name="ps", bufs=2, space="PSUM") as pspool:

            bandt = cpool.tile([128, BANDW + 128], BF16)
            nc.scalar.dma_start(out=bandt[:], in_=band[:, :])
            ident = bandt[:, C0:C0 + 128]
            ident2 = bandt[:, ID2:ID2 + 128]

            for img in range(B_CORE):
                # ---- input DMAs ----
                attf = ipool.tile([128, 2, 4, W], BF16, tag="attf")
                cst = ipool.tile([128, 2, W], BF16, tag="cst")
                cot = ipool.tile([128, 2, W], BF16, tag="cot")
                for b in range(2):
                    nc.scalar.dma_start(out=attf[:, b], in_=att[img, b])
                    nc.scalar.dma_start(out=cst[:, b], in_=cs[img, b])
                    nc.scalar.dma_start(out=cot[:, b], in_=co[img, b])
                zt = zpool.tile([128, 2, NTAP, PW], BF16, tag="zt")
                for (lo, hi, _r) in ZCHUNKS:
                    for b in range(2):
                        nc.sync.dma_start(out=zt[:, b, lo:hi, :],
                                          in_=aff[img, b, :, lo:hi, :])

                # ---- PSUM accumulators: [128, 2(block), W], 1 bank each ----
                psU = pspool.tile([128, 2, W], FP32, tag="U")
                psD = pspool.tile([128, 2, W], FP32, tag="D")
                psT = pspool.tile([128, 2, W], FP32, tag="T")
                started = set()

                def mm(acc, stop=False, **kw):
                    nc.tensor.matmul(start=(acc not in started), stop=stop,
                                     **kw)
                    started.add(acc)

                stg = spool.tile([128, 2, 21, W], BF16, tag="stg")
                rab = spool.tile([128, 2, 14, W], BF16, tag="rab")
                rt = rpool.tile([128, 7, 2, W], BF16, tag="rt")
                dts = []

                for ci, (lo, hi, rings) in enumerate(ZCHUNKS):
                    # z = att_r * aff (DVE, in place, bf16 2x)
                    for b in range(2):
                        for r in rings:
                            rl, rh = max(lo, RING_OF[r][0]), min(hi, RING_OF[r][1])
                            zwin = zt[:, b, rl:rh, GW:GW + W]
                            nc.vector.tensor_tensor(
                                out=zwin, in0=zwin,
                                in1=attf[:, b, r:r + 1, :].broadcast_to(
                                    [128, rh - rl, W]),
                                op=TT.mult)
                    # T butterfly level-1 instrs that become ready:
                    if ci == 1:
                        # L1a: E1..E5 = z[7,9,..,15] + z[8,10,..,16] -> stg 0:5
                        for b in range(2):
                            nc.vector.tensor_tensor(
                                out=_pap(stg, b, 21, 0, [(1, 5)]),
                                in0=_zap(zt, b, 7, [(2, 0, 5)]),
                                in1=_zap(zt, b, 8, [(2, 0, 5)]),
                                op=TT.add)
                        # L1f: P0a..c,P6a..c = pairs of rows 0,6 -> stg 15:21
                        for b in range(2):
                            nc.vector.tensor_tensor(
                                out=_pap(stg, b, 21, 15, [(3, 2), (1, 3)]),
                                in0=_zap(zt, b, 0, [(17, 0, 2), (2, -2, 3)]),
                                in1=_zap(zt, b, 1, [(17, 0, 2), (2, -2, 3)]),
                                op=TT.add)
                    if ci == 2:
                        # L1b: F2,F3,F4 = z[29,31,33]+z[30,32,34] -> stg 5:8
                        for b in range(2):
                            nc.vector.tensor_tensor(
                                out=_pap(stg, b, 21, 5, [(1, 3)]),
                                in0=_zap(zt, b, 29, [(2, 0, 3)]),
                                in1=_zap(zt, b, 30, [(2, 0, 3)]),
                                op=TT.add)
                        # L1c: G1a,G1b,G5a,G5b -> stg 8:12
                        for b in range(2):
                            nc.vector.tensor_tensor(
                                out=_pap(stg, b, 21, 8, [(2, 2), (1, 2)]),
                                in0=_zap(zt, b, 24, [(11, 0, 2), (1, -1, 2)]),
                                in1=_zap(zt, b, 26, [(11, 0, 2), (1, -1, 2)]),
                                op=TT.add)
                    if ci == 3:
                        # L1d: H2,H4 = z[40,45]+z[41,46] -> stg 12:14
                        for b in range(2):
                            nc.vector.tensor_tensor(
                                out=_pap(stg, b, 21, 12, [(1, 2)]),
                                in0=_zap(zt, b, 40, [(5, 0, 2)]),
                                in1=_zap(zt, b, 41, [(5, 0, 2)]),
                                op=TT.add)
                        # L1e: B3 = z43 + z44 -> rab plane 8
                        for b in range(2):
                            nc.vector.tensor_tensor(
                                out=_pap(rab, b, 14, 8, []),
                                in0=_zap(zt, b, 43, []),
                                in1=_zap(zt, b, 44, []),
                                op=TT.add)

                    # U matmuls for this chunk (both blocks paired in rhs)
                    zbase = zt[:]
                    for t in range(lo, hi):
                        mm("U", out=psU[:], lhsT=ident,
                           rhs=AP(zbase.tensor, zbase.offset + t * PW + GW,
                                  [[zbase.ap[0][0], 128], [NTAP * PW, 2],
                                   [1, W]]))
                    # relu -> dt chunks fully covered by z so far
                    zdone = hi
                    for (dlo, dhi) in DCHUNKS:
                        if dlo < hi and dhi <= zdone and (dlo, dhi) not in \
                                [c for c, _ in dts]:
                            dtc = dpool.tile([128, 2, 12, W], BF16, tag="dt")
                            nc.scalar.activation(
                                dtc[:], zt[:, :, dlo:dhi, GW:GW + W],
                                mybir.ActivationFunctionType.Relu,
                                scale=-1.0)
                            dts.append(((dlo, dhi), dtc))
                            # D matmuls for this dt chunk
                            for k in range(dhi - dlo):
                                last = (dhi == NTAP and k == 12 - 1)
                                mm("D", stop=last, out=psD[:], lhsT=ident2,
                                   rhs=dtc[:, :, k, :])

                # ---- butterfly L2/L3 -> rt [128, 7(slot), 2(block), W] ----
                for b in range(2):
                    # L2a1: A2,A3,A4 = E+F rows 2,3,4 -> rab 0:3
                    nc.vector.tensor_tensor(
                        out=_pap(rab, b, 14, 0, [(1, 3)]),
                        in0=_pap(stg, b, 21, 1, [(1, 3)]),
                        in1=_pap(stg, b, 21, 5, [(1, 3)]),
                        op=TT.add)
                    # L2a2: A1,A5 = E1+G1a, E5+G5a -> rab 3:5
                    nc.vector.tensor_tensor(
                        out=_pap(rab, b, 14, 3, [(1, 2)]),
                        in0=_pap(stg, b, 21, 0, [(4, 2)]),
                        in1=_pap(stg, b, 21, 8, [(2, 2)]),
                        op=TT.add)
                    # L2b4: A0,A6 = P0a+P0b, P6a+P6b -> rab 5:7
                    nc.vector.tensor_tensor(
                        out=_pap(rab, b, 14, 5, [(1, 2)]),
                        in0=_pap(stg, b, 21, 15, [(3, 2)]),
                        in1=_pap(stg, b, 21, 16, [(3, 2)]),
                        op=TT.add)
                    # L2b2: B2,B4 = H2+z42, H4+z47 -> rab 7,9
                    nc.vector.tensor_tensor(
                        out=_pap(rab, b, 14, 7, [(2, 2)]),
                        in0=_pap(stg, b, 21, 12, [(1, 2)]),
                        in1=_zap(zt, b, 42, [(5, 0, 2)]),
                        op=TT.add)
                    # L2b1: B1,B5 = G1b+z28, G5b+z39 -> rab 10:12
                    nc.vector.tensor_tensor(
                        out=_pap(rab, b, 14, 10, [(1, 2)]),
                        in0=_pap(stg, b, 21, 9, [(2, 2)]),
                        in1=_zap(zt, b, 28, [(11, 0, 2)]),
                        op=TT.add)
                    # L2b3: B0,B6 = P0c+z6, P6c+z23 -> rab 12:14
                    nc.vector.tensor_tensor(
                        out=_pap(rab, b, 14, 12, [(1, 2)]),
                        in0=_pap(stg, b, 21, 17, [(3, 2)]),
                        in1=_zap(zt, b, 6, [(17, 0, 2)]),
                        op=TT.add)
                    # L3: rt[:, s, b, :] = A_s + B_s
                    base = rt[:]
                    nc.vector.tensor_tensor(
                        out=AP(base.tensor, base.offset + b * W,
                               [[base.ap[0][0], 128], [2 * W, 7], [1, W]]),
                        in0=_pap(rab, b, 14, 0, [(1, 7)]),
                        in1=_pap(rab, b, 14, 7, [(1, 7)]),
                        op=TT.add)

                # ---- T row-shift matmuls ----
                for s, dy in enumerate(SLOT_DY):
                    lw = bandt[:, C0 + dy:C0 + dy + 128]
                    mm("T", out=psT[:], lhsT=lw, rhs=rt[:, s, :, :])
                for s, dy in enumerate(SLOT_DY):
                    if dy == 0:
                        continue
                    if dy > 0:
                        hw_ = bandt[:, 3 + dy:3 + dy + 128]
                        nc.tensor.matmul(start=False, stop=False,
                                         out=psT[:, 0:1, :], lhsT=hw_,
                                         rhs=rt[:, s, 1:2, :])
                    else:
                        hw_ = bandt[:, 259 + dy:259 + dy + 128]
                        nc.tensor.matmul(start=False, stop=False,
                                         out=psT[:, 1:2, :], lhsT=hw_,
                                         rhs=rt[:, s, 0:1, :])

                # closers: psU += att3, psT += att3 (stop their groups)
                mm("U", stop=True, out=psU[:], lhsT=ident,
                   rhs=attf[:, :, 3, :])
                mm("T", stop=True, out=psT[:], lhsT=ident,
                   rhs=attf[:, :, 3, :])

                # ---- epilogue (both blocks per instr) ----
                dsb = epool.tile([128, 2, W], FP32, tag="dsb")
                nc.scalar.copy(dsb[:], psD[:])
                et = epool.tile([128, 2, W], FP32, tag="et")
                nc.vector.scalar_tensor_tensor(
                    out=et[:], in0=psU[:], scalar=EPS, in1=dsb[:],
                    op0=TT.add, op1=TT.add)
                rcp = epool.tile([128, 2, W], FP32, tag="rcp")
                nc.vector.reciprocal_approx_fast(out=rcp[:], in_=et[:])
                n1 = epool.tile([128, 2, W], FP32, tag="n1")
                nc.vector.tensor_tensor(out=n1[:], in0=psT[:], in1=cst[:],
                                        op=TT.mult)
                n2 = epool.tile([128, 2, W], FP32, tag="n2")
                nc.vector.tensor_tensor(out=n2[:], in0=dsb[:], in1=cot[:],
                                        op=TT.mult)
                nc.vector.tensor_tensor(out=n1[:], in0=n1[:], in1=n2[:],
                                        op=TT.add)
                nc.vector.tensor_tensor(out=n1[:], in0=n1[:], in1=rcp[:],
                                        op=TT.mult)
                for b in range(2):
                    nc.sync.dma_start(
                        out=out[img, 0, b * 128:b * 128 + 128, :],
                        in_=n1[:, b, :])

    nc.compile()
    return nc


_NC_CACHE = None


def _get_nc():
    global _NC_CACHE
    if _NC_CACHE is None:
        _NC_CACHE = _build()
    return _NC_CACHE


def run(inputs: dict, trace: bool = False):
    aff = np.ascontiguousarray(np.asarray(inputs["affinity"], dtype=np.float32))
    att = np.ascontiguousarray(np.asarray(inputs["attention"], dtype=np.float32))
    cs = np.ascontiguousarray(
        np.asarray(inputs["current_segmentation"], dtype=np.float32))
    co = np.ascontiguousarray(
        np.asarray(inputs["coarse_segmentation"], dtype=np.float32))
    in_maps = pack_inputs(aff, att, cs, co)

    nc = _get_nc()
    last_err = None
    for attempt in range(3):
        try:
            res = run_bass_kernel_spmd(nc, in_maps, list(range(N_CORES)),
                                       trace=trace)
            break
        except Exception as e:
            last_err = e
            import time
            time.sleep(10)
    else:
        raise last_err
    full = np.concatenate([res.results[c]["out"] for c in range(N_CORES)],
                          axis=0)
    return full, res


def kernel(**inputs) -> np.ndarray:
    out, _ = run(inputs, trace=False)
    return out


# revision 3
# speedup vs baseline: 1.0317x; 1.0075x over previous
"""DYSPN attention-conv kernel v3 for Trainium2 (8 NeuronCores, batch-parallel).

Same math as v2, restructured to cut PE work ~40% and rebalance engines:
  per image, tap k=(i,j) != center, ring r = INDEX[i,j], dy = 3-i, dx = 3-j:
    z_k[y,x] = att_r[y,x] * aff_k[y,x]
    U[y,x]   = sum_k z_k[y,x]
    D[y,x]   = sum_k 2*relu(-z_k)[y,x]
    T[y,x]   = sum_k z_k[y+dy, x+dx]  (zero outside image)
  out = ((T + att3)*cs + D*co) / (U + D + att3 + eps)

v3 layout/strategy:
  - z tile holds BOTH 128-row blocks [128, 2, 48, 264]; U/D matmuls pair the
    two blocks in the rhs free dim (512-wide) so psU/psD/psT are one PSUM
    bank each ([128, 2(block), 256]).
  - T: per-row column-shifted sums R_i built on DVE (bf16 2x) via a 3-level
    butterfly (13 instrs/block, affine APs), then only 7 main + 6 halo
    row-shift matmuls per image on PE (vs 90 tap matmuls in v2).
  - D: relu(-z) on ACT into chunked dt tiles, reduced with ident2 matmuls.
  - cs/co shipped as bf16 (halves their HBM traffic).
"""
import sys

sys.path.insert(0, "/opt/trn_rl_repo")

import numpy as np
import ml_dtypes

import concourse.bass as bass  # noqa: F401
import concourse.tile as tile
from concourse import bacc, mybir
from concourse.ap import AP
from concourse.bass_utils import run_bass_kernel_spmd

BF16 = mybir.dt.bfloat16
FP32 = mybir.dt.float32

N_CORES = 8
B_FULL = 16
B_CORE = B_FULL // N_CORES
H = W = 256
K = 7
GW = 4                    # zero guard cols each side (host-packed)
PW = W + 2 * GW           # 264: plane pitch in the z tile
NTAP = 48
BANDW = 390
C0 = 131                  # identity diagonal column offset in band1
ID2 = BANDW               # ident2 (2.0 diagonal) starts at this column
EPS = 1e-6

_INDEX = np.array([0, 0, 0, 0, 0, 0, 0,
                   0, 1, 1, 1, 1, 1, 0,
                   0, 1, 2, 2, 2, 1, 0,
                   0, 1, 2, 3, 2, 1, 0,
                   0, 1, 2, 2, 2, 1, 0,
                   0, 1, 1, 1, 1, 1, 0,
                   0, 0, 0, 0, 0, 0, 0], dtype=np.int64).reshape(K, K)

# ring-major, row-minor, col-minor tap order (same as v2 packing)
TAPORD = [(i, j) for r in (0, 1, 2) for i in range(K) for j in range(K)
          if (i, j) != (3, 3) and _INDEX[i, j] == r]

# z-mult chunks (ring-aligned) and relu/dt chunks (uniform 12)
ZCHUNKS = [(0, 12, (0,)), (12, 24, (0,)), (24, 40, (1,)), (40, 48, (2,))]
RING_OF = {0: (0, 24), 1: (24, 40), 2: (40, 48)}
DCHUNKS = [(0, 12), (12, 24), (24, 36), (36, 48)]

# T row-slot order produced by the butterfly: rows [2,3,4,1,5,0,6]
SLOT_ROWS = [2, 3, 4, 1, 5, 0, 6]
SLOT_DY = [3 - i for i in SLOT_ROWS]   # [1, 0, -1, 2, -2, 3, -3]


def dxof(t):
    return 3 - TAPORD[t][1]


def band_np() -> np.ndarray:
    b = np.zeros((128, BANDW + 128), dtype=np.float32)
    for p in range(128):
        b[p, p + C0] = 1.0
        b[p, ID2 + p] = 2.0
    return b.astype(ml_dtypes.bfloat16)


def _to_bf16_round(x: np.ndarray) -> np.ndarray:
    """fp32 -> bf16 with round-to-nearest-even, fast numpy path."""
    u = x.view(np.uint32)
    r = ((u >> 16) & 1).astype(np.uint32)
    out = ((u + 0x7FFF + r) >> 16).astype(np.uint16)
    return out.view(ml_dtypes.bfloat16)


def pack_inputs(aff, att, cs, co):
    """Full fp32 inputs -> per-core input maps (host-side layout/cast only)."""
    B = B_FULL
    kidx = np.array([i * K + j for (i, j) in TAPORD])
    aff_sel = aff[:, kidx]                             # [B,48,H,W]
    aff_bf = _to_bf16_round(np.ascontiguousarray(aff_sel))
    packed = np.zeros((B, 2, 128, NTAP, PW), dtype=ml_dtypes.bfloat16)
    a = aff_bf.reshape(B, NTAP, 2, 128, W).transpose(0, 2, 3, 1, 4)
    packed[:, :, :, :, GW:GW + W] = a
    att_bf = _to_bf16_round(np.ascontiguousarray(att))  # [B,4,H,W]
    att_p = att_bf.reshape(B, 4, 2, 128, W).transpose(0, 2, 3, 1, 4)
    att_p = np.ascontiguousarray(att_p)                 # [B,2,128,4,W]
    cs_bf = _to_bf16_round(
        np.ascontiguousarray(cs, dtype=np.float32)).reshape(B, 2, 128, W)
    co_bf = _to_bf16_round(
        np.ascontiguousarray(co, dtype=np.float32)).reshape(B, 2, 128, W)
    band = band_np()

    in_maps = []
    for c in range(N_CORES):
        s = slice(c * B_CORE, (c + 1) * B_CORE)
        in_maps.append({
            "aff": np.ascontiguousarray(packed[s]),
            "att": np.ascontiguousarray(att_p[s]),
            "cs": np.ascontiguousarray(cs_bf[s]),
            "co": np.ascontiguousarray(co_bf[s]),
            "band": band,
        })
    return in_maps


def _zap(zt, b, t0, dims):
    """AP over the z tile: start at tap t0 (window GW+dxof(t0)), free dims
    described as (dt, ddx, n) pairs (tap step + window-offset step), with a
    final [1, W] column run."""
    base = zt[:]
    pstride = base.ap[0][0]
    off = base.offset + b * NTAP * PW + t0 * PW + GW + dxof(t0)
    ap = [[pstride, 128]] + [[dt * PW + ddx, n] for (dt, ddx, n) in dims] \
        + [[1, W]]
    return AP(base.tensor, off, ap)


def _pap(tt, b, nplanes, p0, dims):
    """AP over a [128, 2, nplanes, W] staging tile."""
    base = tt[:]
    pstride = base.ap[0][0]
    off = base.offset + b * nplanes * W + p0 * W
    ap = [[pstride, 128]] + [[dp * W, n] for (dp, n) in dims] + [[1, W]]
    return AP(base.tensor, off, ap)


def _zapB(zt, t0, dims):
    """Like _zap but spanning both blocks as the leading free dim."""
    base = zt[:]
    pstride = base.ap[0][0]
    off = base.offset + t0 * PW + GW + dxof(t0)
    ap = [[pstride, 128], [NTAP * PW, 2]] \
        + [[dt * PW + ddx, n] for (dt, ddx, n) in dims] + [[1, W]]
    return AP(base.tensor, off, ap)


def _papB(tt, nplanes, p0, dims):
    """Like _pap but spanning both blocks as the leading free dim."""
    base = tt[:]
    pstride = base.ap[0][0]
    off = base.offset + p0 * W
    ap = [[pstride, 128], [nplanes * W, 2]] \
        + [[dp * W, n] for (dp, n) in dims] + [[1, W]]
    return AP(base.tensor, off, ap)


def _build():
    nc = bacc.Bacc("TRN2", target_bir_lowering=False, debug=False,
                   num_devices=N_CORES)
    aff = nc.dram_tensor("aff", [B_CORE, 2, 128, NTAP, PW], BF16,
                         kind="ExternalInput").ap()
    att = nc.dram_tensor("att", [B_CORE, 2, 128, 4, W], BF16,
                         kind="ExternalInput").ap()
    cs = nc.dram_tensor("cs", [B_CORE, 2, 128, W], BF16,
                        kind="ExternalInput").ap()
    co = nc.dram_tensor("co", [B_CORE, 2, 128, W], BF16,
                        kind="ExternalInput").ap()
    band = nc.dram_tensor("band", [128, BANDW + 128], BF16,
                          kind="ExternalInput").ap()
    out = nc.dram_tensor("out", [B_CORE, 1, H, W], FP32,
                         kind="ExternalOutput").ap()

    TT = mybir.AluOpType
    with tile.TileContext(nc) as tc:
        with tc.tile_pool(name="const", bufs=1) as cpool, \
             tc.tile_pool(name="inp", bufs=2) as ipool, \
             tc.tile_pool(name="zp", bufs=2) as zpool, \
             tc.tile_pool(name="dp", bufs=2) as dpool, \
             tc.tile_pool(name="st", bufs=1) as spool, \
             tc.tile_pool(name="rp", bufs=2) as rpool, \
             tc.tile_pool(name="ep", bufs=1) as epool, \
             tc.tile_pool(name="ps", bufs=2, space="PSUM") as pspool:

            bandt = cpool.tile([128, BANDW + 128], BF16)
            nc.scalar.dma_start(out=bandt[:], in_=band[:, :])
            ident = bandt[:, C0:C0 + 128]
            ident2 = bandt[:, ID2:ID2 + 128]

            for img in range(B_CORE):
                # ---- input DMAs ----
                attf = ipool.tile([128, 2, 4, W], BF16, tag="attf")
                cst = ipool.tile([128, 2, W], BF16, tag="cst")
                cot = ipool.tile([128, 2, W], BF16, tag="cot")
                for b in range(2):
                    nc.sync.dma_start(out=attf[:, b], in_=att[img, b])
                zt = zpool.tile([128, 2, NTAP, PW], BF16, tag="zt")
                for (lo, hi, _r) in ZCHUNKS:
                    nc.sync.dma_start(out=zt[:, 0, lo:hi, :],
                                      in_=aff[img, 0, :, lo:hi, :])
                    nc.sync.dma_start(out=zt[:, 1, lo:hi, :],
                                      in_=aff[img, 1, :, lo:hi, :])
                for b in range(2):
                    nc.sync.dma_start(out=cst[:, b], in_=cs[img, b])
                    nc.sync.dma_start(out=cot[:, b], in_=co[img, b])

                # ---- PSUM accumulators: [128, 2(block), W], 1 bank each ----
                psU = pspool.tile([128, 2, W], FP32, tag="U")
                psD = pspool.tile([128, 2, W], FP32, tag="D")
                psT = pspool.tile([128, 2, W], FP32, tag="T")
                started = set()

                def mm(acc, stop=False, **kw):
                    nc.tensor.matmul(start=(acc not in started), stop=stop,
                                     **kw)
                    started.add(acc)

                stg = spool.tile([128, 2, 21, W], BF16, tag="stg")
                rab = spool.tile([128, 2, 14, W], BF16, tag="rab")
                rt = rpool.tile([128, 7, 2, W], BF16, tag="rt")
                dts = []

                d_pending = []

                def flush_d():
                    while d_pending:
                        dtc, k, last = d_pending.pop(0)
                        mm("D", stop=last, out=psD[:], lhsT=ident2,
                           rhs=dtc[:, :, k, :])

                for ci, (lo, hi, rings) in enumerate(ZCHUNKS):
                    # z = att_r * aff (DVE, in place, bf16 2x); chunk 0 is
                    # split per block so compute starts on first arrival
                    for r in rings:
                        rl, rh = max(lo, RING_OF[r][0]), min(hi, RING_OF[r][1])
                        if ci == 0:
                            for b in range(2):
                                zwin = zt[:, b, rl:rh, GW:GW + W]
                                nc.vector.tensor_tensor(
                                    out=zwin, in0=zwin,
                                    in1=attf[:, b, r:r + 1, :].broadcast_to(
                                        [128, rh - rl, W]),
                                    op=TT.mult)
                        else:
                            zwin = zt[:, :, rl:rh, GW:GW + W]
                            nc.vector.tensor_tensor(
                                out=zwin, in0=zwin,
                                in1=attf[:, :, r:r + 1, :].broadcast_to(
                                    [128, 2, rh - rl, W]),
                                op=TT.mult)
                    # T butterfly level-1 instrs that become ready:
                    if ci == 1:
                        # L1a: E1..E5 = z[7,9,..,15] + z[8,10,..,16] -> stg 0:5
                        nc.vector.tensor_tensor(
                            out=_papB(stg, 21, 0, [(1, 5)]),
                            in0=_zapB(zt, 7, [(2, 0, 5)]),
                            in1=_zapB(zt, 8, [(2, 0, 5)]),
                            op=TT.add)
                        # L1f: P0a..c,P6a..c = pairs of rows 0,6 -> stg 15:21
                        for b in range(2):
                            nc.vector.tensor_tensor(
                                out=_pap(stg, b, 21, 15, [(3, 2), (1, 3)]),
                                in0=_zap(zt, b, 0, [(17, 0, 2), (2, -2, 3)]),
                                in1=_zap(zt, b, 1, [(17, 0, 2), (2, -2, 3)]),
                                op=TT.add)
                    if ci == 2:
                        # L1b: F2,F3,F4 = z[29,31,33]+z[30,32,34] -> stg 5:8
                        nc.vector.tensor_tensor(
                            out=_papB(stg, 21, 5, [(1, 3)]),
                            in0=_zapB(zt, 29, [(2, 0, 3)]),
                            in1=_zapB(zt, 30, [(2, 0, 3)]),
                            op=TT.add)
                        # L1c: G1a,G1b,G5a,G5b -> stg 8:12
                        for b in range(2):
                            nc.vector.tensor_tensor(
                                out=_pap(stg, b, 21, 8, [(2, 2), (1, 2)]),
                                in0=_zap(zt, b, 24, [(11, 0, 2), (1, -1, 2)]),
                                in1=_zap(zt, b, 26, [(11, 0, 2), (1, -1, 2)]),
                                op=TT.add)
                    if ci == 3:
                        # L1d: H2,H4 = z[40,45]+z[41,46] -> stg 12:14
                        nc.vector.tensor_tensor(
                            out=_papB(stg, 21, 12, [(1, 2)]),
                            in0=_zapB(zt, 40, [(5, 0, 2)]),
                            in1=_zapB(zt, 41, [(5, 0, 2)]),
                            op=TT.add)
                        # L1e: B3 = z43 + z44 -> rab plane 8
                        nc.vector.tensor_tensor(
                            out=_papB(rab, 14, 8, []),
                            in0=_zapB(zt, 43, []),
                            in1=_zapB(zt, 44, []),
                            op=TT.add)

                    # U matmuls for this chunk (both blocks paired in rhs)
                    zbase = zt[:]
                    for t in range(lo, hi):
                        mm("U", out=psU[:], lhsT=ident,
                           rhs=AP(zbase.tensor, zbase.offset + t * PW + GW,
                                  [[zbase.ap[0][0], 128], [NTAP * PW, 2],
                                   [1, W]]))
                    # relu -> dt chunks fully covered by z so far (tap-halved
                    # for finer ACT->PE handoff); D matmuls queue until after
                    # the next U run so PE keeps long same-weight runs
                    zdone = hi
                    for (dlo, dhi) in DCHUNKS:
                        if dlo < hi and dhi <= zdone and (dlo, dhi) not in \
                                [c for c, _ in dts]:
                            dtc = dpool.tile([128, 2, 12, W], BF16, tag="dt")
                            for (hlo, hhi) in ((0, 6), (6, 12)):
                                nc.scalar.activation(
                                    dtc[:, :, hlo:hhi, :],
                                    zt[:, :, dlo + hlo:dlo + hhi, GW:GW + W],
                                    mybir.ActivationFunctionType.Relu,
                                    scale=-1.0)
                                for k in range(hlo, hhi):
                                    last = (dhi == NTAP and k == 11)
                                    d_pending.append((dtc, k, last))
                            dts.append(((dlo, dhi), dtc))
                    if ci == 1 or ci == 3:
                        flush_d()

                # ---- butterfly L2/L3 -> rt [128, 7(slot), 2(block), W] ----
                # L2a1: A2,A3,A4 = E+F rows 2,3,4 -> rab 0:3
                nc.vector.tensor_tensor(
                    out=_papB(rab, 14, 0, [(1, 3)]),
                    in0=_papB(stg, 21, 1, [(1, 3)]),
                    in1=_papB(stg, 21, 5, [(1, 3)]),
                    op=TT.add)
                # L2a2: A1,A5 = E1+G1a, E5+G5a -> rab 3:5
                nc.vector.tensor_tensor(
                    out=_papB(rab, 14, 3, [(1, 2)]),
                    in0=_papB(stg, 21, 0, [(4, 2)]),
                    in1=_papB(stg, 21, 8, [(2, 2)]),
                    op=TT.add)
                # L2b4: A0,A6 = P0a+P0b, P6a+P6b -> rab 5:7
                nc.vector.tensor_tensor(
                    out=_papB(rab, 14, 5, [(1, 2)]),
                    in0=_papB(stg, 21, 15, [(3, 2)]),
                    in1=_papB(stg, 21, 16, [(3, 2)]),
                    op=TT.add)
                # L2b2: B2,B4 = H2+z42, H4+z47 -> rab 7,9
                nc.vector.tensor_tensor(
                    out=_papB(rab, 14, 7, [(2, 2)]),
                    in0=_papB(stg, 21, 12, [(1, 2)]),
                    in1=_zapB(zt, 42, [(5, 0, 2)]),
                    op=TT.add)
                # L2b1: B1,B5 = G1b+z28, G5b+z39 -> rab 10:12
                nc.vector.tensor_tensor(
                    out=_papB(rab, 14, 10, [(1, 2)]),
                    in0=_papB(stg, 21, 9, [(2, 2)]),
                    in1=_zapB(zt, 28, [(11, 0, 2)]),
                    op=TT.add)
                # L2b3: B0,B6 = P0c+z6, P6c+z23 -> rab 12:14
                nc.vector.tensor_tensor(
                    out=_papB(rab, 14, 12, [(1, 2)]),
                    in0=_papB(stg, 21, 17, [(3, 2)]),
                    in1=_zapB(zt, 6, [(17, 0, 2)]),
                    op=TT.add)
                # L3: rt[:, s, b, :] = A_s + B_s  (both blocks in one instr)
                base = rt[:]
                nc.vector.tensor_tensor(
                    out=AP(base.tensor, base.offset,
                           [[base.ap[0][0], 128], [W, 2], [2 * W, 7],
                            [1, W]]),
                    in0=_papB(rab, 14, 0, [(1, 7)]),
                    in1=_papB(rab, 14, 7, [(1, 7)]),
                    op=TT.add)

                # ---- T row-shift matmuls ----
                for s, dy in enumerate(SLOT_DY):
                    lw = bandt[:, C0 + dy:C0 + dy + 128]
                    mm("T", out=psT[:], lhsT=lw, rhs=rt[:, s, :, :])
                for s, dy in enumerate(SLOT_DY):
                    if dy == 0:
                        continue
                    if dy > 0:
                        hw_ = bandt[:, 3 + dy:3 + dy + 128]
                        nc.tensor.matmul(start=False, stop=False,
                                         out=psT[:, 0:1, :], lhsT=hw_,
                                         rhs=rt[:, s, 1:2, :])
                    else:
                        hw_ = bandt[:, 259 + dy:259 + dy + 128]
                        nc.tensor.matmul(start=False, stop=False,
                                         out=psT[:, 1:2, :], lhsT=hw_,
                                         rhs=rt[:, s, 0:1, :])

                # closers: psU += att3, psT += att3 (stop their groups)
                mm("U", stop=True, out=psU[:], lhsT=ident,
                   rhs=attf[:, :, 3, :])
                mm("T", stop=True, out=psT[:], lhsT=ident,
                   rhs=attf[:, :, 3, :])

                # ---- epilogue (both blocks per instr) ----
                dsb = epool.tile([128, 2, W], FP32, tag="dsb")
                nc.scalar.copy(dsb[:], psD[:])
                et = epool.tile([128, 2, W], FP32, tag="et")
                nc.vector.scalar_tensor_tensor(
                    out=et[:], in0=psU[:], scalar=EPS, in1=dsb[:],
                    op0=TT.add, op1=TT.add)
                rcp = epool.tile([128, 2, W], FP32, tag="rcp")
                nc.vector.reciprocal_approx_fast(out=rcp[:], in_=et[:])
                n1 = epool.tile([128, 2, W], FP32, tag="n1")
                nc.vector.tensor_tensor(out=n1[:], in0=psT[:], in1=cst[:],
                                        op=TT.mult)
                n2 = epool.tile([128, 2, W], FP32, tag="n2")
                nc.vector.tensor_tensor(out=n2[:], in0=dsb[:], in1=cot[:],
                                        op=TT.mult)
                nc.vector.tensor_tensor(out=n1[:], in0=n1[:], in1=n2[:],
                                        op=TT.add)
                nc.vector.tensor_tensor(out=n1[:], in0=n1[:], in1=rcp[:],
                                        op=TT.mult)
                for b in range(2):
                    nc.sync.dma_start(
                        out=out[img, 0, b * 128:b * 128 + 128, :],
                        in_=n1[:, b, :])

    nc.compile()
    return nc


_NC_CACHE = None


def _get_nc():
    global _NC_CACHE
    if _NC_CACHE is None:
        _NC_CACHE = _build()
    return _NC_CACHE


def run(inputs: dict, trace: bool = False):
    aff = np.ascontiguousarray(np.asarray(inputs["affinity"], dtype=np.float32))
    att = np.ascontiguousarray(np.asarray(inputs["attention"], dtype=np.float32))
    cs = np.ascontiguousarray(
        np.asarray(inputs["current_segmentation"], dtype=np.float32))
    co = np.ascontiguousarray(
        np.asarray(inputs["coarse_segmentation"], dtype=np.float32))
    in_maps = pack_inputs(aff, att, cs, co)

    nc = _get_nc()
    last_err = None
    for attempt in range(3):
        try:
            res = run_bass_kernel_spmd(nc, in_maps, list(range(N_CORES)),
                                       trace=trace)
            break
        except Exception as e:
            last_err = e
            import time
            time.sleep(10)
    else:
        raise last_err
    full = np.concatenate([res.results[c]["out"] for c in range(N_CORES)],
                          axis=0)
    return full, res


def kernel(**inputs) -> np.ndarray:
    out, _ = run(inputs, trace=False)
    return out


# revision 4
# speedup vs baseline: 1.0573x; 1.0249x over previous
"""DYSPN attention-conv kernel v3 for Trainium2 (8 NeuronCores, batch-parallel).

Same math as v2, restructured to cut PE work ~40% and rebalance engines:
  per image, tap k=(i,j) != center, ring r = INDEX[i,j], dy = 3-i, dx = 3-j:
    z_k[y,x] = att_r[y,x] * aff_k[y,x]
    U[y,x]   = sum_k z_k[y,x]
    D[y,x]   = sum_k 2*relu(-z_k)[y,x]
    T[y,x]   = sum_k z_k[y+dy, x+dx]  (zero outside image)
  out = ((T + att3)*cs + D*co) / (U + D + att3 + eps)

v3 layout/strategy:
  - z tile holds BOTH 128-row blocks [128, 2, 48, 264]; U/D matmuls pair the
    two blocks in the rhs free dim (512-wide) so psU/psD/psT are one PSUM
    bank each ([128, 2(block), 256]).
  - T: per-row column-shifted sums R_i built on DVE (bf16 2x) via a 3-level
    butterfly (13 instrs/block, affine APs), then only 7 main + 6 halo
    row-shift matmuls per image on PE (vs 90 tap matmuls in v2).
  - D: relu(-z) on ACT into chunked dt tiles, reduced with ident2 matmuls.
  - cs/co shipped as bf16 (halves their HBM traffic).
"""
import sys

sys.path.insert(0, "/opt/trn_rl_repo")

import numpy as np
import ml_dtypes

import concourse.bass as bass  # noqa: F401
import concourse.tile as tile
from concourse import bacc, mybir
from concourse.ap import AP
from concourse.bass_utils import run_bass_kernel_spmd

BF16 = mybir.dt.bfloat16
FP32 = mybir.dt.float32

N_CORES = 8
B_FULL = 16
B_CORE = B_FULL // N_CORES
H = W = 256
K = 7
GW = 4                    # zero guard cols each side (host-packed)
PW = W + 2 * GW           # 264: plane pitch in the z tile
NTAP = 48
BANDW = 390
C0 = 131                  # identity diagonal column offset in band1
ID2 = BANDW               # ident2 (2.0 diagonal) starts at this column
EPS = 1e-6

_INDEX = np.array([0, 0, 0, 0, 0, 0, 0,
                   0, 1, 1, 1, 1, 1, 0,
                   0, 1, 2, 2, 2, 1, 0,
                   0, 1, 2, 3, 2, 1, 0,
                   0, 1, 2, 2, 2, 1, 0,
                   0, 1, 1, 1, 1, 1, 0,
                   0, 0, 0, 0, 0, 0, 0], dtype=np.int64).reshape(K, K)

# ring-major, row-minor, col-minor tap order (same as v2 packing)
TAPORD = [(i, j) for r in (0, 1, 2) for i in range(K) for j in range(K)
          if (i, j) != (3, 3) and _INDEX[i, j] == r]

# z-mult chunks (ring-aligned; tiny first chunk so compute starts early)
ZCHUNKS = [(0, 7, (0,)), (7, 24, (0,)), (24, 40, (1,)), (40, 48, (2,))]
RING_OF = {0: (0, 24), 1: (24, 40), 2: (40, 48)}
DCHUNKS = [(0, 12), (12, 24), (24, 36), (36, 48)]

# T row-slot order produced by the butterfly: rows [2,3,4,1,5,0,6]
SLOT_ROWS = [2, 3, 4, 1, 5, 0, 6]
SLOT_DY = [3 - i for i in SLOT_ROWS]   # [1, 0, -1, 2, -2, 3, -3]


def dxof(t):
    return 3 - TAPORD[t][1]


def band_np() -> np.ndarray:
    b = np.zeros((128, BANDW + 128), dtype=np.float32)
    for p in range(128):
        b[p, p + C0] = 1.0
        b[p, ID2 + p] = 2.0
    return b.astype(ml_dtypes.bfloat16)


def _to_bf16_round(x: np.ndarray) -> np.ndarray:
    """fp32 -> bf16 with round-to-nearest-even, fast numpy path."""
    u = x.view(np.uint32)
    r = ((u >> 16) & 1).astype(np.uint32)
    out = ((u + 0x7FFF + r) >> 16).astype(np.uint16)
    return out.view(ml_dtypes.bfloat16)


def pack_inputs(aff, att, cs, co):
    """Full fp32 inputs -> per-core input maps (host-side layout/cast only)."""
    B = B_FULL
    kidx = np.array([i * K + j for (i, j) in TAPORD])
    aff_sel = aff[:, kidx]                             # [B,48,H,W]
    aff_bf = _to_bf16_round(np.ascontiguousarray(aff_sel))
    packed = np.zeros((B, 2, 128, NTAP, PW), dtype=ml_dtypes.bfloat16)
    a = aff_bf.reshape(B, NTAP, 2, 128, W).transpose(0, 2, 3, 1, 4)
    packed[:, :, :, :, GW:GW + W] = a
    att_bf = _to_bf16_round(np.ascontiguousarray(att))  # [B,4,H,W]
    att_p = att_bf.reshape(B, 4, 2, 128, W).transpose(0, 2, 3, 1, 4)
    att_p = np.ascontiguousarray(att_p)                 # [B,2,128,4,W]
    cs_bf = _to_bf16_round(
        np.ascontiguousarray(cs, dtype=np.float32)).reshape(B, 2, 128, W)
    co_bf = _to_bf16_round(
        np.ascontiguousarray(co, dtype=np.float32)).reshape(B, 2, 128, W)
    band = band_np()

    in_maps = []
    for c in range(N_CORES):
        s = slice(c * B_CORE, (c + 1) * B_CORE)
        in_maps.append({
            "aff": np.ascontiguousarray(packed[s]),
            "att": np.ascontiguousarray(att_p[s]),
            "cs": np.ascontiguousarray(cs_bf[s]),
            "co": np.ascontiguousarray(co_bf[s]),
            "band": band,
        })
    return in_maps


def _zap(zt, b, t0, dims):
    """AP over the z tile: start at tap t0 (window GW+dxof(t0)), free dims
    described as (dt, ddx, n) pairs (tap step + window-offset step), with a
    final [1, W] column run."""
    base = zt[:]
    pstride = base.ap[0][0]
    off = base.offset + b * NTAP * PW + t0 * PW + GW + dxof(t0)
    ap = [[pstride, 128]] + [[dt * PW + ddx, n] for (dt, ddx, n) in dims] \
        + [[1, W]]
    return AP(base.tensor, off, ap)


def _pap(tt, b, nplanes, p0, dims):
    """AP over a [128, 2, nplanes, W] staging tile."""
    base = tt[:]
    pstride = base.ap[0][0]
    off = base.offset + b * nplanes * W + p0 * W
    ap = [[pstride, 128]] + [[dp * W, n] for (dp, n) in dims] + [[1, W]]
    return AP(base.tensor, off, ap)


def _zapB(zt, t0, dims):
    """Like _zap but spanning both blocks as the leading free dim."""
    base = zt[:]
    pstride = base.ap[0][0]
    off = base.offset + t0 * PW + GW + dxof(t0)
    ap = [[pstride, 128], [NTAP * PW, 2]] \
        + [[dt * PW + ddx, n] for (dt, ddx, n) in dims] + [[1, W]]
    return AP(base.tensor, off, ap)


def _papB(tt, nplanes, p0, dims):
    """Like _pap but spanning both blocks as the leading free dim."""
    base = tt[:]
    pstride = base.ap[0][0]
    off = base.offset + p0 * W
    ap = [[pstride, 128], [nplanes * W, 2]] \
        + [[dp * W, n] for (dp, n) in dims] + [[1, W]]
    return AP(base.tensor, off, ap)


def _build():
    nc = bacc.Bacc("TRN2", target_bir_lowering=False, debug=False,
                   num_devices=N_CORES)
    aff = nc.dram_tensor("aff", [B_CORE, 2, 128, NTAP, PW], BF16,
                         kind="ExternalInput").ap()
    att = nc.dram_tensor("att", [B_CORE, 2, 128, 4, W], BF16,
                         kind="ExternalInput").ap()
    cs = nc.dram_tensor("cs", [B_CORE, 2, 128, W], BF16,
                        kind="ExternalInput").ap()
    co = nc.dram_tensor("co", [B_CORE, 2, 128, W], BF16,
                        kind="ExternalInput").ap()
    band = nc.dram_tensor("band", [128, BANDW + 128], BF16,
                          kind="ExternalInput").ap()
    out = nc.dram_tensor("out", [B_CORE, 1, H, W], FP32,
                         kind="ExternalOutput").ap()

    TT = mybir.AluOpType
    with tile.TileContext(nc) as tc:
        with tc.tile_pool(name="const", bufs=1) as cpool, \
             tc.tile_pool(name="inp", bufs=2) as ipool, \
             tc.tile_pool(name="zp", bufs=2) as zpool, \
             tc.tile_pool(name="dp", bufs=2) as dpool, \
             tc.tile_pool(name="st", bufs=1) as spool, \
             tc.tile_pool(name="rp", bufs=2) as rpool, \
             tc.tile_pool(name="ep", bufs=1) as epool, \
             tc.tile_pool(name="ps", bufs=2, space="PSUM") as pspool:

            bandt = cpool.tile([128, BANDW + 128], BF16)
            nc.scalar.dma_start(out=bandt[:], in_=band[:, :])
            ident = bandt[:, C0:C0 + 128]
            ident2 = bandt[:, ID2:ID2 + 128]

            pending_epi = []

            def run_pending():
                while pending_epi:
                    pending_epi.pop(0)()

            for img in range(B_CORE):
                # ---- input DMAs ----
                attf = ipool.tile([128, 2, 4, W], BF16, tag="attf")
                cst = ipool.tile([128, 2, W], BF16, tag="cst")
                cot = ipool.tile([128, 2, W], BF16, tag="cot")
                for b in range(2):
                    nc.sync.dma_start(out=attf[:, b], in_=att[img, b])
                zt = zpool.tile([128, 2, NTAP, PW], BF16, tag="zt")
                for (lo, hi, _r) in ZCHUNKS:
                    nc.sync.dma_start(out=zt[:, 0, lo:hi, :],
                                      in_=aff[img, 0, :, lo:hi, :])
                    nc.sync.dma_start(out=zt[:, 1, lo:hi, :],
                                      in_=aff[img, 1, :, lo:hi, :])
                for b in range(2):
                    nc.sync.dma_start(out=cst[:, b], in_=cs[img, b])
                    nc.sync.dma_start(out=cot[:, b], in_=co[img, b])

                # ---- PSUM accumulators: [128, 2(block), W], 1 bank each ----
                psU = pspool.tile([128, 2, W], FP32, tag="U")
                psD = pspool.tile([128, 2, W], FP32, tag="D")
                psT = pspool.tile([128, 2, W], FP32, tag="T")
                started = set()

                def mm(acc, stop=False, **kw):
                    nc.tensor.matmul(start=(acc not in started), stop=stop,
                                     **kw)
                    started.add(acc)

                stg = spool.tile([128, 2, 21, W], BF16, tag="stg")
                rab = spool.tile([128, 2, 14, W], BF16, tag="rab")
                rt = rpool.tile([128, 7, 2, W], BF16, tag="rt")
                dts = []

                d_pending = []

                def flush_d():
                    while d_pending:
                        dtc, k, last = d_pending.pop(0)
                        mm("D", stop=last, out=psD[:], lhsT=ident2,
                           rhs=dtc[:, :, k, :])

                for ci, (lo, hi, rings) in enumerate(ZCHUNKS):
                    # z = att_r * aff (DVE, in place, bf16 2x); chunk 0 is
                    # split per block so compute starts on first arrival
                    for r in rings:
                        rl, rh = max(lo, RING_OF[r][0]), min(hi, RING_OF[r][1])
                        if ci == 0:
                            for b in range(2):
                                zwin = zt[:, b, rl:rh, GW:GW + W]
                                nc.vector.tensor_tensor(
                                    out=zwin, in0=zwin,
                                    in1=attf[:, b, r:r + 1, :].broadcast_to(
                                        [128, rh - rl, W]),
                                    op=TT.mult)
                        else:
                            zwin = zt[:, :, rl:rh, GW:GW + W]
                            nc.vector.tensor_tensor(
                                out=zwin, in0=zwin,
                                in1=attf[:, :, r:r + 1, :].broadcast_to(
                                    [128, 2, rh - rl, W]),
                                op=TT.mult)
                    # T butterfly level-1 instrs that become ready:
                    if ci == 1:
                        # L1a: E1..E5 = z[7,9,..,15] + z[8,10,..,16] -> stg 0:5
                        nc.vector.tensor_tensor(
                            out=_papB(stg, 21, 0, [(1, 5)]),
                            in0=_zapB(zt, 7, [(2, 0, 5)]),
                            in1=_zapB(zt, 8, [(2, 0, 5)]),
                            op=TT.add)
                        # L1f: P0a..c,P6a..c = pairs of rows 0,6 -> stg 15:21
                        for b in range(2):
                            nc.vector.tensor_tensor(
                                out=_pap(stg, b, 21, 15, [(3, 2), (1, 3)]),
                                in0=_zap(zt, b, 0, [(17, 0, 2), (2, -2, 3)]),
                                in1=_zap(zt, b, 1, [(17, 0, 2), (2, -2, 3)]),
                                op=TT.add)
                    if ci == 2:
                        # L1b: F2,F3,F4 = z[29,31,33]+z[30,32,34] -> stg 5:8
                        nc.vector.tensor_tensor(
                            out=_papB(stg, 21, 5, [(1, 3)]),
                            in0=_zapB(zt, 29, [(2, 0, 3)]),
                            in1=_zapB(zt, 30, [(2, 0, 3)]),
                            op=TT.add)
                        # L1c: G1a,G1b,G5a,G5b -> stg 8:12
                        for b in range(2):
                            nc.vector.tensor_tensor(
                                out=_pap(stg, b, 21, 8, [(2, 2), (1, 2)]),
                                in0=_zap(zt, b, 24, [(11, 0, 2), (1, -1, 2)]),
                                in1=_zap(zt, b, 26, [(11, 0, 2), (1, -1, 2)]),
                                op=TT.add)
                    if ci == 3:
                        # L1d: H2,H4 = z[40,45]+z[41,46] -> stg 12:14
                        nc.vector.tensor_tensor(
                            out=_papB(stg, 21, 12, [(1, 2)]),
                            in0=_zapB(zt, 40, [(5, 0, 2)]),
                            in1=_zapB(zt, 41, [(5, 0, 2)]),
                            op=TT.add)
                        # L1e: B3 = z43 + z44 -> rab plane 8
                        nc.vector.tensor_tensor(
                            out=_papB(rab, 14, 8, []),
                            in0=_zapB(zt, 43, []),
                            in1=_zapB(zt, 44, []),
                            op=TT.add)

                    # U matmuls for this chunk (both blocks paired in rhs)
                    zbase = zt[:]
                    for t in range(lo, hi):
                        mm("U", out=psU[:], lhsT=ident,
                           rhs=AP(zbase.tensor, zbase.offset + t * PW + GW,
                                  [[zbase.ap[0][0], 128], [NTAP * PW, 2],
                                   [1, W]]))
                    # relu -> dt chunks fully covered by z so far (tap-halved
                    # for finer ACT->PE handoff); D matmuls queue until after
                    # the next U run so PE keeps long same-weight runs
                    zdone = hi
                    for (dlo, dhi) in DCHUNKS:
                        if dlo < hi and dhi <= zdone and (dlo, dhi) not in \
                                [c for c, _ in dts]:
                            dtc = dpool.tile([128, 2, 12, W], BF16, tag="dt")
                            for (hlo, hhi) in ((0, 6), (6, 12)):
                                nc.scalar.activation(
                                    dtc[:, :, hlo:hhi, :],
                                    zt[:, :, dlo + hlo:dlo + hhi, GW:GW + W],
                                    mybir.ActivationFunctionType.Relu,
                                    scale=-1.0)
                                for k in range(hlo, hhi):
                                    last = (dhi == NTAP and k == 11)
                                    d_pending.append((dtc, k, last))
                            dts.append(((dlo, dhi), dtc))
                    if ci == 1 or ci == 3:
                        flush_d()

                # previous image's deferred epilogue: PE has caught up by
                # now, so DVE no longer stalls on its PSUM stops
                run_pending()

                # ---- butterfly L2/L3 -> rt [128, 7(slot), 2(block), W] ----
                # L2a1: A2,A3,A4 = E+F rows 2,3,4 -> rab 0:3
                nc.vector.tensor_tensor(
                    out=_papB(rab, 14, 0, [(1, 3)]),
                    in0=_papB(stg, 21, 1, [(1, 3)]),
                    in1=_papB(stg, 21, 5, [(1, 3)]),
                    op=TT.add)
                # L2a2: A1,A5 = E1+G1a, E5+G5a -> rab 3:5
                nc.vector.tensor_tensor(
                    out=_papB(rab, 14, 3, [(1, 2)]),
                    in0=_papB(stg, 21, 0, [(4, 2)]),
                    in1=_papB(stg, 21, 8, [(2, 2)]),
                    op=TT.add)
                # L2b4: A0,A6 = P0a+P0b, P6a+P6b -> rab 5:7
                nc.vector.tensor_tensor(
                    out=_papB(rab, 14, 5, [(1, 2)]),
                    in0=_papB(stg, 21, 15, [(3, 2)]),
                    in1=_papB(stg, 21, 16, [(3, 2)]),
                    op=TT.add)
                # L2b2: B2,B4 = H2+z42, H4+z47 -> rab 7,9
                nc.vector.tensor_tensor(
                    out=_papB(rab, 14, 7, [(2, 2)]),
                    in0=_papB(stg, 21, 12, [(1, 2)]),
                    in1=_zapB(zt, 42, [(5, 0, 2)]),
                    op=TT.add)
                # L2b1: B1,B5 = G1b+z28, G5b+z39 -> rab 10:12
                nc.vector.tensor_tensor(
                    out=_papB(rab, 14, 10, [(1, 2)]),
                    in0=_papB(stg, 21, 9, [(2, 2)]),
                    in1=_zapB(zt, 28, [(11, 0, 2)]),
                    op=TT.add)
                # L2b3: B0,B6 = P0c+z6, P6c+z23 -> rab 12:14
                nc.vector.tensor_tensor(
                    out=_papB(rab, 14, 12, [(1, 2)]),
                    in0=_papB(stg, 21, 17, [(3, 2)]),
                    in1=_zapB(zt, 6, [(17, 0, 2)]),
                    op=TT.add)
                # L3: rt[:, s, b, :] = A_s + B_s  (both blocks in one instr)
                base = rt[:]
                nc.vector.tensor_tensor(
                    out=AP(base.tensor, base.offset,
                           [[base.ap[0][0], 128], [W, 2], [2 * W, 7],
                            [1, W]]),
                    in0=_papB(rab, 14, 0, [(1, 7)]),
                    in1=_papB(rab, 14, 7, [(1, 7)]),
                    op=TT.add)

                # ---- T row-shift matmuls ----
                for s, dy in enumerate(SLOT_DY):
                    lw = bandt[:, C0 + dy:C0 + dy + 128]
                    mm("T", out=psT[:], lhsT=lw, rhs=rt[:, s, :, :])
                for s, dy in enumerate(SLOT_DY):
                    if dy == 0:
                        continue
                    if dy > 0:
                        hw_ = bandt[:, 3 + dy:3 + dy + 128]
                        nc.tensor.matmul(start=False, stop=False,
                                         out=psT[:, 0:1, :], lhsT=hw_,
                                         rhs=rt[:, s, 1:2, :])
                    else:
                        hw_ = bandt[:, 259 + dy:259 + dy + 128]
                        nc.tensor.matmul(start=False, stop=False,
                                         out=psT[:, 1:2, :], lhsT=hw_,
                                         rhs=rt[:, s, 0:1, :])

                # closers: psU += att3, psT += att3 (stop their groups)
                mm("U", stop=True, out=psU[:], lhsT=ident,
                   rhs=attf[:, :, 3, :])
                mm("T", stop=True, out=psT[:], lhsT=ident,
                   rhs=attf[:, :, 3, :])

                # ---- epilogue (deferred: DVE ops run after the next
                # image's front so DVE never waits on this image's PE) ----
                dsb = epool.tile([128, 2, W], FP32, tag="dsb")
                nc.scalar.copy(dsb[:], psD[:])

                def epi(img=img, psU=psU, psT=psT, dsb=dsb, cst=cst, cot=cot):
                    et = epool.tile([128, 2, W], FP32, tag="et")
                    nc.vector.scalar_tensor_tensor(
                        out=et[:], in0=psU[:], scalar=EPS, in1=dsb[:],
                        op0=TT.add, op1=TT.add)
                    rcp = epool.tile([128, 2, W], FP32, tag="rcp")
                    nc.vector.reciprocal_approx_fast(out=rcp[:], in_=et[:])
                    n1 = epool.tile([128, 2, W], FP32, tag="n1")
                    nc.vector.tensor_tensor(out=n1[:], in0=psT[:],
                                            in1=cst[:], op=TT.mult)
                    n2 = epool.tile([128, 2, W], FP32, tag="n2")
                    nc.vector.tensor_tensor(out=n2[:], in0=dsb[:],
                                            in1=cot[:], op=TT.mult)
                    nc.vector.tensor_tensor(out=n1[:], in0=n1[:], in1=n2[:],
                                            op=TT.add)
                    nc.vector.tensor_tensor(out=n1[:], in0=n1[:], in1=rcp[:],
                                            op=TT.mult)
                    for b in range(2):
                        nc.sync.dma_start(
                            out=out[img, 0, b * 128:b * 128 + 128, :],
                            in_=n1[:, b, :])

                pending_epi.append(epi)

            run_pending()

    nc.compile()
    return nc


_NC_CACHE = None


def _get_nc():
    global _NC_CACHE
    if _NC_CACHE is None:
        _NC_CACHE = _build()
    return _NC_CACHE


def run(inputs: dict, trace: bool = False):
    aff = np.ascontiguousarray(np.asarray(inputs["affinity"], dtype=np.float32))
    att = np.ascontiguousarray(np.asarray(inputs["attention"], dtype=np.float32))
    cs = np.ascontiguousarray(
        np.asarray(inputs["current_segmentation"], dtype=np.float32))
    co = np.ascontiguousarray(
        np.asarray(inputs["coarse_segmentation"], dtype=np.float32))
    in_maps = pack_inputs(aff, att, cs, co)

    nc = _get_nc()
    last_err = None
    for attempt in range(3):
        try:
            res = run_bass_kernel_spmd(nc, in_maps, list(range(N_CORES)),
                                       trace=trace)
            break
        except Exception as e:
            last_err = e
            import time
            time.sleep(10)
    else:
        raise last_err
    full = np.concatenate([res.results[c]["out"] for c in range(N_CORES)],
                          axis=0)
    return full, res


def kernel(**inputs) -> np.ndarray:
    out, _ = run(inputs, trace=False)
    return out


# revision 5
# speedup vs baseline: 1.0628x; 1.0051x over previous
"""DYSPN attention-conv kernel v3 for Trainium2 (8 NeuronCores, batch-parallel).

Same math as v2, restructured to cut PE work ~40% and rebalance engines:
  per image, tap k=(i,j) != center, ring r = INDEX[i,j], dy = 3-i, dx = 3-j:
    z_k[y,x] = att_r[y,x] * aff_k[y,x]
    U[y,x]   = sum_k z_k[y,x]
    D[y,x]   = sum_k 2*relu(-z_k)[y,x]
    T[y,x]   = sum_k z_k[y+dy, x+dx]  (zero outside image)
  out = ((T + att3)*cs + D*co) / (U + D + att3 + eps)

v3 layout/strategy:
  - z tile holds BOTH 128-row blocks [128, 2, 48, 264]; U/D matmuls pair the
    two blocks in the rhs free dim (512-wide) so psU/psD/psT are one PSUM
    bank each ([128, 2(block), 256]).
  - T: per-row column-shifted sums R_i built on DVE (bf16 2x) via a 3-level
    butterfly (13 instrs/block, affine APs), then only 7 main + 6 halo
    row-shift matmuls per image on PE (vs 90 tap matmuls in v2).
  - D: relu(-z) on ACT into chunked dt tiles, reduced with ident2 matmuls.
  - cs/co shipped as bf16 (halves their HBM traffic).
"""
import sys

sys.path.insert(0, "/opt/trn_rl_repo")

import numpy as np
import ml_dtypes

import concourse.bass as bass  # noqa: F401
import concourse.tile as tile
from concourse import bacc, mybir
from concourse.ap import AP
from concourse.bass_utils import run_bass_kernel_spmd

BF16 = mybir.dt.bfloat16
FP32 = mybir.dt.float32

N_CORES = 8
B_FULL = 16
B_CORE = B_FULL // N_CORES
H = W = 256
K = 7
GW = 4                    # zero guard cols each side (host-packed)
PW = W + 2 * GW           # 264: plane pitch in the z tile
NTAP = 48
BANDW = 390
C0 = 131                  # identity diagonal column offset in band1
ID2 = BANDW               # ident2 (2.0 diagonal) starts at this column
EPS = 1e-6

_INDEX = np.array([0, 0, 0, 0, 0, 0, 0,
                   0, 1, 1, 1, 1, 1, 0,
                   0, 1, 2, 2, 2, 1, 0,
                   0, 1, 2, 3, 2, 1, 0,
                   0, 1, 2, 2, 2, 1, 0,
                   0, 1, 1, 1, 1, 1, 0,
                   0, 0, 0, 0, 0, 0, 0], dtype=np.int64).reshape(K, K)

# ring-major, row-minor, col-minor tap order (same as v2 packing)
TAPORD = [(i, j) for r in (0, 1, 2) for i in range(K) for j in range(K)
          if (i, j) != (3, 3) and _INDEX[i, j] == r]

# z-mult chunks (ring-aligned; small early chunks so compute starts early
# and DVE never waits long for the next DMA)
ZCHUNKS = [(0, 7, (0,)), (7, 16, (0,)), (16, 24, (0,)), (24, 40, (1,)),
           (40, 48, (2,))]
RING_OF = {0: (0, 24), 1: (24, 40), 2: (40, 48)}
DCHUNKS = [(0, 12), (12, 24), (24, 36), (36, 48)]

# T row-slot order produced by the butterfly: rows [2,3,4,1,5,0,6]
SLOT_ROWS = [2, 3, 4, 1, 5, 0, 6]
SLOT_DY = [3 - i for i in SLOT_ROWS]   # [1, 0, -1, 2, -2, 3, -3]


def dxof(t):
    return 3 - TAPORD[t][1]


def band_np() -> np.ndarray:
    b = np.zeros((128, BANDW + 128), dtype=np.float32)
    for p in range(128):
        b[p, p + C0] = 1.0
        b[p, ID2 + p] = 2.0
    return b.astype(ml_dtypes.bfloat16)


def _to_bf16_round(x: np.ndarray) -> np.ndarray:
    """fp32 -> bf16 with round-to-nearest-even, fast numpy path."""
    u = x.view(np.uint32)
    r = ((u >> 16) & 1).astype(np.uint32)
    out = ((u + 0x7FFF + r) >> 16).astype(np.uint16)
    return out.view(ml_dtypes.bfloat16)


def pack_inputs(aff, att, cs, co):
    """Full fp32 inputs -> per-core input maps (host-side layout/cast only)."""
    B = B_FULL
    kidx = np.array([i * K + j for (i, j) in TAPORD])
    aff_sel = aff[:, kidx]                             # [B,48,H,W]
    aff_bf = _to_bf16_round(np.ascontiguousarray(aff_sel))
    packed = np.zeros((B, 2, 128, NTAP, PW), dtype=ml_dtypes.bfloat16)
    a = aff_bf.reshape(B, NTAP, 2, 128, W).transpose(0, 2, 3, 1, 4)
    packed[:, :, :, :, GW:GW + W] = a
    att_bf = _to_bf16_round(np.ascontiguousarray(att))  # [B,4,H,W]
    att_p = att_bf.reshape(B, 4, 2, 128, W).transpose(0, 2, 3, 1, 4)
    att_p = np.ascontiguousarray(att_p)                 # [B,2,128,4,W]
    cs_bf = _to_bf16_round(
        np.ascontiguousarray(cs, dtype=np.float32)).reshape(B, 2, 128, W)
    co_bf = _to_bf16_round(
        np.ascontiguousarray(co, dtype=np.float32)).reshape(B, 2, 128, W)
    band = band_np()

    in_maps = []
    for c in range(N_CORES):
        s = slice(c * B_CORE, (c + 1) * B_CORE)
        in_maps.append({
            "aff": np.ascontiguousarray(packed[s]),
            "att": np.ascontiguousarray(att_p[s]),
            "cs": np.ascontiguousarray(cs_bf[s]),
            "co": np.ascontiguousarray(co_bf[s]),
            "band": band,
        })
    return in_maps


def _zap(zt, b, t0, dims):
    """AP over the z tile: start at tap t0 (window GW+dxof(t0)), free dims
    described as (dt, ddx, n) pairs (tap step + window-offset step), with a
    final [1, W] column run."""
    base = zt[:]
    pstride = base.ap[0][0]
    off = base.offset + b * NTAP * PW + t0 * PW + GW + dxof(t0)
    ap = [[pstride, 128]] + [[dt * PW + ddx, n] for (dt, ddx, n) in dims] \
        + [[1, W]]
    return AP(base.tensor, off, ap)


def _pap(tt, b, nplanes, p0, dims):
    """AP over a [128, 2, nplanes, W] staging tile."""
    base = tt[:]
    pstride = base.ap[0][0]
    off = base.offset + b * nplanes * W + p0 * W
    ap = [[pstride, 128]] + [[dp * W, n] for (dp, n) in dims] + [[1, W]]
    return AP(base.tensor, off, ap)


def _zapB(zt, t0, dims):
    """Like _zap but spanning both blocks as the leading free dim."""
    base = zt[:]
    pstride = base.ap[0][0]
    off = base.offset + t0 * PW + GW + dxof(t0)
    ap = [[pstride, 128], [NTAP * PW, 2]] \
        + [[dt * PW + ddx, n] for (dt, ddx, n) in dims] + [[1, W]]
    return AP(base.tensor, off, ap)


def _papB(tt, nplanes, p0, dims):
    """Like _pap but spanning both blocks as the leading free dim."""
    base = tt[:]
    pstride = base.ap[0][0]
    off = base.offset + p0 * W
    ap = [[pstride, 128], [nplanes * W, 2]] \
        + [[dp * W, n] for (dp, n) in dims] + [[1, W]]
    return AP(base.tensor, off, ap)


def _build():
    nc = bacc.Bacc("TRN2", target_bir_lowering=False, debug=False,
                   num_devices=N_CORES)
    aff = nc.dram_tensor("aff", [B_CORE, 2, 128, NTAP, PW], BF16,
                         kind="ExternalInput").ap()
    att = nc.dram_tensor("att", [B_CORE, 2, 128, 4, W], BF16,
                         kind="ExternalInput").ap()
    cs = nc.dram_tensor("cs", [B_CORE, 2, 128, W], BF16,
                        kind="ExternalInput").ap()
    co = nc.dram_tensor("co", [B_CORE, 2, 128, W], BF16,
                        kind="ExternalInput").ap()
    band = nc.dram_tensor("band", [128, BANDW + 128], BF16,
                          kind="ExternalInput").ap()
    out = nc.dram_tensor("out", [B_CORE, 1, H, W], FP32,
                         kind="ExternalOutput").ap()

    TT = mybir.AluOpType
    with tile.TileContext(nc) as tc:
        with tc.tile_pool(name="const", bufs=1) as cpool, \
             tc.tile_pool(name="inp", bufs=2) as ipool, \
             tc.tile_pool(name="zp", bufs=2) as zpool, \
             tc.tile_pool(name="dp", bufs=2) as dpool, \
             tc.tile_pool(name="st", bufs=1) as spool, \
             tc.tile_pool(name="rp", bufs=2) as rpool, \
             tc.tile_pool(name="ep", bufs=1) as epool, \
             tc.tile_pool(name="ps", bufs=2, space="PSUM") as pspool:

            bandt = cpool.tile([128, BANDW + 128], BF16)
            nc.scalar.dma_start(out=bandt[:], in_=band[:, :])
            ident = bandt[:, C0:C0 + 128]
            ident2 = bandt[:, ID2:ID2 + 128]

            pending_epi = []

            def run_pending():
                while pending_epi:
                    pending_epi.pop(0)()

            for img in range(B_CORE):
                # ---- input DMAs ----
                attf = ipool.tile([128, 2, 4, W], BF16, tag="attf")
                cst = ipool.tile([128, 2, W], BF16, tag="cst")
                cot = ipool.tile([128, 2, W], BF16, tag="cot")
                for b in range(2):
                    nc.sync.dma_start(out=attf[:, b], in_=att[img, b])
                zt = zpool.tile([128, 2, NTAP, PW], BF16, tag="zt")
                for (lo, hi, _r) in ZCHUNKS:
                    nc.sync.dma_start(out=zt[:, 0, lo:hi, :],
                                      in_=aff[img, 0, :, lo:hi, :])
                    nc.sync.dma_start(out=zt[:, 1, lo:hi, :],
                                      in_=aff[img, 1, :, lo:hi, :])
                for b in range(2):
                    nc.sync.dma_start(out=cst[:, b], in_=cs[img, b])
                    nc.sync.dma_start(out=cot[:, b], in_=co[img, b])

                # ---- PSUM accumulators: [128, 2(block), W], 1 bank each ----
                psU = pspool.tile([128, 2, W], FP32, tag="U")
                psD = pspool.tile([128, 2, W], FP32, tag="D")
                psT = pspool.tile([128, 2, W], FP32, tag="T")
                started = set()

                def mm(acc, stop=False, **kw):
                    nc.tensor.matmul(start=(acc not in started), stop=stop,
                                     **kw)
                    started.add(acc)

                stg = spool.tile([128, 2, 21, W], BF16, tag="stg")
                rab = spool.tile([128, 2, 14, W], BF16, tag="rab")
                rt = rpool.tile([128, 7, 2, W], BF16, tag="rt")
                dts = []

                d_pending = []

                def flush_d():
                    while d_pending:
                        dtc, k, last = d_pending.pop(0)
                        mm("D", stop=last, out=psD[:], lhsT=ident2,
                           rhs=dtc[:, :, k, :])

                for ci, (lo, hi, rings) in enumerate(ZCHUNKS):
                    # z = att_r * aff (DVE, in place, bf16 2x); chunk 0 is
                    # split per block so compute starts on first arrival
                    for r in rings:
                        rl, rh = max(lo, RING_OF[r][0]), min(hi, RING_OF[r][1])
                        if ci == 0:
                            for b in range(2):
                                zwin = zt[:, b, rl:rh, GW:GW + W]
                                nc.vector.tensor_tensor(
                                    out=zwin, in0=zwin,
                                    in1=attf[:, b, r:r + 1, :].broadcast_to(
                                        [128, rh - rl, W]),
                                    op=TT.mult)
                        else:
                            zwin = zt[:, :, rl:rh, GW:GW + W]
                            nc.vector.tensor_tensor(
                                out=zwin, in0=zwin,
                                in1=attf[:, :, r:r + 1, :].broadcast_to(
                                    [128, 2, rh - rl, W]),
                                op=TT.mult)
                    # T butterfly level-1 instrs that become ready:
                    if ci == 2:
                        # L1a: E1..E5 = z[7,9,..,15] + z[8,10,..,16] -> stg 0:5
                        nc.vector.tensor_tensor(
                            out=_papB(stg, 21, 0, [(1, 5)]),
                            in0=_zapB(zt, 7, [(2, 0, 5)]),
                            in1=_zapB(zt, 8, [(2, 0, 5)]),
                            op=TT.add)
                        # L1f: P0a..c,P6a..c = pairs of rows 0,6 -> stg 15:21
                        for b in range(2):
                            nc.vector.tensor_tensor(
                                out=_pap(stg, b, 21, 15, [(3, 2), (1, 3)]),
                                in0=_zap(zt, b, 0, [(17, 0, 2), (2, -2, 3)]),
                                in1=_zap(zt, b, 1, [(17, 0, 2), (2, -2, 3)]),
                                op=TT.add)
                    if ci == 3:
                        # L1b: F2,F3,F4 = z[29,31,33]+z[30,32,34] -> stg 5:8
                        nc.vector.tensor_tensor(
                            out=_papB(stg, 21, 5, [(1, 3)]),
                            in0=_zapB(zt, 29, [(2, 0, 3)]),
                            in1=_zapB(zt, 30, [(2, 0, 3)]),
                            op=TT.add)
                        # L1c: G1a,G1b,G5a,G5b -> stg 8:12
                        for b in range(2):
                            nc.vector.tensor_tensor(
                                out=_pap(stg, b, 21, 8, [(2, 2), (1, 2)]),
                                in0=_zap(zt, b, 24, [(11, 0, 2), (1, -1, 2)]),
                                in1=_zap(zt, b, 26, [(11, 0, 2), (1, -1, 2)]),
                                op=TT.add)
                    if ci == 4:
                        # L1d: H2,H4 = z[40,45]+z[41,46] -> stg 12:14
                        nc.vector.tensor_tensor(
                            out=_papB(stg, 21, 12, [(1, 2)]),
                            in0=_zapB(zt, 40, [(5, 0, 2)]),
                            in1=_zapB(zt, 41, [(5, 0, 2)]),
                            op=TT.add)
                        # L1e: B3 = z43 + z44 -> rab plane 8
                        nc.vector.tensor_tensor(
                            out=_papB(rab, 14, 8, []),
                            in0=_zapB(zt, 43, []),
                            in1=_zapB(zt, 44, []),
                            op=TT.add)

                    # U matmuls for this chunk (both blocks paired in rhs)
                    zbase = zt[:]
                    for t in range(lo, hi):
                        mm("U", out=psU[:], lhsT=ident,
                           rhs=AP(zbase.tensor, zbase.offset + t * PW + GW,
                                  [[zbase.ap[0][0], 128], [NTAP * PW, 2],
                                   [1, W]]))
                    # relu -> dt chunks fully covered by z so far (tap-halved
                    # for finer ACT->PE handoff); D matmuls queue until after
                    # the next U run so PE keeps long same-weight runs
                    zdone = hi
                    for (dlo, dhi) in DCHUNKS:
                        if dlo < hi and dhi <= zdone and (dlo, dhi) not in \
                                [c for c, _ in dts]:
                            dtc = dpool.tile([128, 2, 12, W], BF16, tag="dt")
                            for (hlo, hhi) in ((0, 6), (6, 12)):
                                nc.scalar.activation(
                                    dtc[:, :, hlo:hhi, :],
                                    zt[:, :, dlo + hlo:dlo + hhi, GW:GW + W],
                                    mybir.ActivationFunctionType.Relu,
                                    scale=-1.0)
                                for k in range(hlo, hhi):
                                    last = (dhi == NTAP and k == 11)
                                    d_pending.append((dtc, k, last))
                            dts.append(((dlo, dhi), dtc))
                    if ci == 2 or ci == 4:
                        flush_d()

                # previous image's deferred epilogue: PE has caught up by
                # now, so DVE no longer stalls on its PSUM stops
                run_pending()

                # ---- butterfly L2/L3 -> rt [128, 7(slot), 2(block), W] ----
                # L2a1: A2,A3,A4 = E+F rows 2,3,4 -> rab 0:3
                nc.vector.tensor_tensor(
                    out=_papB(rab, 14, 0, [(1, 3)]),
                    in0=_papB(stg, 21, 1, [(1, 3)]),
                    in1=_papB(stg, 21, 5, [(1, 3)]),
                    op=TT.add)
                # L2a2: A1,A5 = E1+G1a, E5+G5a -> rab 3:5
                nc.vector.tensor_tensor(
                    out=_papB(rab, 14, 3, [(1, 2)]),
                    in0=_papB(stg, 21, 0, [(4, 2)]),
                    in1=_papB(stg, 21, 8, [(2, 2)]),
                    op=TT.add)
                # L2b4: A0,A6 = P0a+P0b, P6a+P6b -> rab 5:7
                nc.vector.tensor_tensor(
                    out=_papB(rab, 14, 5, [(1, 2)]),
                    in0=_papB(stg, 21, 15, [(3, 2)]),
                    in1=_papB(stg, 21, 16, [(3, 2)]),
                    op=TT.add)
                # L2b2: B2,B4 = H2+z42, H4+z47 -> rab 7,9
                nc.vector.tensor_tensor(
                    out=_papB(rab, 14, 7, [(2, 2)]),
                    in0=_papB(stg, 21, 12, [(1, 2)]),
                    in1=_zapB(zt, 42, [(5, 0, 2)]),
                    op=TT.add)
                # L2b1: B1,B5 = G1b+z28, G5b+z39 -> rab 10:12
                nc.vector.tensor_tensor(
                    out=_papB(rab, 14, 10, [(1, 2)]),
                    in0=_papB(stg, 21, 9, [(2, 2)]),
                    in1=_zapB(zt, 28, [(11, 0, 2)]),
                    op=TT.add)
                # L2b3: B0,B6 = P0c+z6, P6c+z23 -> rab 12:14
                nc.vector.tensor_tensor(
                    out=_papB(rab, 14, 12, [(1, 2)]),
                    in0=_papB(stg, 21, 17, [(3, 2)]),
                    in1=_zapB(zt, 6, [(17, 0, 2)]),
                    op=TT.add)
                # L3: rt[:, s, b, :] = A_s + B_s  (both blocks in one instr)
                base = rt[:]
                nc.vector.tensor_tensor(
                    out=AP(base.tensor, base.offset,
                           [[base.ap[0][0], 128], [W, 2], [2 * W, 7],
                            [1, W]]),
                    in0=_papB(rab, 14, 0, [(1, 7)]),
                    in1=_papB(rab, 14, 7, [(1, 7)]),
                    op=TT.add)

                # ---- T row-shift matmuls ----
                for s, dy in enumerate(SLOT_DY):
                    lw = bandt[:, C0 + dy:C0 + dy + 128]
                    mm("T", out=psT[:], lhsT=lw, rhs=rt[:, s, :, :])
                for s, dy in enumerate(SLOT_DY):
                    if dy == 0:
                        continue
                    if dy > 0:
                        hw_ = bandt[:, 3 + dy:3 + dy + 128]
                        nc.tensor.matmul(start=False, stop=False,
                                         out=psT[:, 0:1, :], lhsT=hw_,
                                         rhs=rt[:, s, 1:2, :])
                    else:
                        hw_ = bandt[:, 259 + dy:259 + dy + 128]
                        nc.tensor.matmul(start=False, stop=False,
                                         out=psT[:, 1:2, :], lhsT=hw_,
                                         rhs=rt[:, s, 0:1, :])

                # closers: psU += att3, psT += att3 (stop their groups)
                mm("U", stop=True, out=psU[:], lhsT=ident,
                   rhs=attf[:, :, 3, :])
                mm("T", stop=True, out=psT[:], lhsT=ident,
                   rhs=attf[:, :, 3, :])

                # ---- epilogue (deferred: DVE ops run after the next
                # image's front so DVE never waits on this image's PE) ----
                dsb = epool.tile([128, 2, W], FP32, tag="dsb")
                nc.scalar.copy(dsb[:], psD[:])

                def epi(img=img, psU=psU, psT=psT, dsb=dsb, cst=cst, cot=cot):
                    et = epool.tile([128, 2, W], FP32, tag="et")
                    nc.vector.scalar_tensor_tensor(
                        out=et[:], in0=psU[:], scalar=EPS, in1=dsb[:],
                        op0=TT.add, op1=TT.add)
                    rcp = epool.tile([128, 2, W], FP32, tag="rcp")
                    nc.vector.reciprocal_approx_fast(out=rcp[:], in_=et[:])
                    n1 = epool.tile([128, 2, W], FP32, tag="n1")
                    nc.vector.tensor_tensor(out=n1[:], in0=psT[:],
                                            in1=cst[:], op=TT.mult)
                    n2 = epool.tile([128, 2, W], FP32, tag="n2")
                    nc.vector.tensor_tensor(out=n2[:], in0=dsb[:],
                                            in1=cot[:], op=TT.mult)
                    nc.vector.tensor_tensor(out=n1[:], in0=n1[:], in1=n2[:],
                                            op=TT.add)
                    nc.vector.tensor_tensor(out=n1[:], in0=n1[:], in1=rcp[:],
                                            op=TT.mult)
                    for b in range(2):
                        nc.sync.dma_start(
                            out=out[img, 0, b * 128:b * 128 + 128, :],
                            in_=n1[:, b, :])

                pending_epi.append(epi)

            run_pending()

    nc.compile()
    return nc


_NC_CACHE = None


def _get_nc():
    global _NC_CACHE
    if _NC_CACHE is None:
        _NC_CACHE = _build()
    return _NC_CACHE


def run(inputs: dict, trace: bool = False):
    aff = np.ascontiguousarray(np.asarray(inputs["affinity"], dtype=np.float32))
    att = np.ascontiguousarray(np.asarray(inputs["attention"], dtype=np.float32))
    cs = np.ascontiguousarray(
        np.asarray(inputs["current_segmentation"], dtype=np.float32))
    co = np.ascontiguousarray(
        np.asarray(inputs["coarse_segmentation"], dtype=np.float32))
    in_maps = pack_inputs(aff, att, cs, co)

    nc = _get_nc()
    last_err = None
    for attempt in range(3):
        try:
            res = run_bass_kernel_spmd(nc, in_maps, list(range(N_CORES)),
                                       trace=trace)
            break
        except Exception as e:
            last_err = e
            import time
            time.sleep(10)
    else:
        raise last_err
    full = np.concatenate([res.results[c]["out"] for c in range(N_CORES)],
                          axis=0)
    return full, res


def kernel(**inputs) -> np.ndarray:
    out, _ = run(inputs, trace=False)
    return out
